# revision 1
# baseline (speedup 1.0000x reference)
"""Trainium2 Bass kernel for nn_AutoregressivePrior (8-slot LSTM prior).

Strategy: pure data-parallel over batch (16384 rows -> 2048 per NeuronCore),
weights replicated. Feature-major dataflow on chip: every activation lives as
[feature_partition, batch_free] so LSTM matmul chains never transpose.
Matmuls in bf16 (PSUM accumulates fp32), gate nonlinearities fused with bias
on ScalarE straight out of PSUM, cell state held in fp16 in SBUF.

Per core the 2048-row batch is processed in two sequential passes of 1024
columns so all state (Wh 8MB bf16 + h/c/x/eps/...) fits in SBUF.

Inputs arrive as full-size numpy arrays; outputs are returned full-size
(zs, mus, sigmas) each [num_slots, 16384, 256] fp32, matching the reference.
"""

import sys

if "/opt/trn_rl_repo" not in sys.path:
    sys.path.insert(0, "/opt/trn_rl_repo")

import numpy as np
import ml_dtypes

BF16 = ml_dtypes.bfloat16

B = 16384
N_CORES = 8
B_LOC = B // N_CORES  # 2048
SCENE = 256
FEAT = 256
HID = 1024
G4 = 4 * HID  # 4096
NS = 8
P = 128
PO = FEAT // P  # 2
KH = HID // P  # 8
KX = FEAT // P  # 2
MT = G4 // P  # 32
HM = (2 * FEAT) // P  # 4 head m-tiles (mu 2 + softplus 2)

PASS_W = 1024  # batch columns per pass on chip

_PATCHED = False


def _patch_tile_drain():
    """walrus in this toolchain rejects >1 sync-wait on a single instruction;
    split excess waits onto standalone single-wait EventSemaphore instructions
    that run on the same engine immediately before the original instruction."""
    global _PATCHED
    if _PATCHED:
        return
    import bass_rust
    import concourse.tile as tile
    from concourse import mybir
    from concourse.vector_clock import ScopedClock

    MAXW = 1
    _orig_lower = tile.TileContext._lower_ordered_insts

    def _lower_split_waits(self, ordered):
        nc = self.nc
        for bbn, insts in ordered.items():
            out = []
            for inst in insts:
                si = getattr(inst, "sync_info", None)
                if si is not None:
                    waits = list(si.on_wait)
                    if len(waits) > MAXW:
                        imm = [w for w in waits if w.wait_mode == "sem-ge-imm"]
                        other = [w for w in waits if w.wait_mode != "sem-ge-imm"]
                        assert len(other) <= MAXW, (inst.name, waits)
                        keep_n = MAXW - len(other)
                        if keep_n > 0:
                            move = imm[: len(imm) - keep_n]
                            keep = imm[len(imm) - keep_n :]
                        else:
                            move = imm
                            keep = []
                        for wt in move:
                            wi = mybir.InstEventSemaphore(
                                name=nc.get_next_instruction_name(),
                                ins=[],
                                outs=[],
                                engine=inst.engine,
                            )
                            wi.sync_info = bass_rust.SyncInfo(
                                on_wait=[wt], on_update=[]
                            )
                            out.append(wi)
                        si.on_wait = other + keep
                out.append(inst)
            insts[:] = out
        return _orig_lower(self, ordered)

    tile.TileContext._lower_ordered_insts = _lower_split_waits

    def _drain_and_barrier(self, tick_clock, wait_clock):
        nc = self.nc
        drain_inst = nc.sync.drain()
        wait_clock.add_sem_waits(
            drain_inst.ins, ScopedClock({None: tick_clock.global_clock})
        )
        si = drain_inst.ins.sync_info
        if si is not None and len(si.on_wait) > 1:
            waits = list(si.on_wait)
            si.on_wait = waits[:1]
            name2handle = {h.name: h for h in self.sems.allocated().values()}
            for w in waits[1:]:
                assert w.wait_mode == "sem-ge-imm", w
                nc.sync.wait_ge(name2handle[w.ant_name], w.wait_value)
        nc.all_engine_barrier()
        popped = nc._tile_sem_poison_stack.pop()
        assert popped is self._sem_poison
        nc.clear_and_free_semaphores(list(self.sems.allocated().values()))
        nc.all_engine_barrier()

    tile.TileContext._drain_and_barrier = _drain_and_barrier
    _PATCHED = True


def build(b_loc=B_LOC, w=PASS_W, n_slots=NS, mm_n=512):
    _patch_tile_drain()
    import concourse.bass as bass
    import concourse.tile as tile
    from concourse import mybir

    F32 = mybir.dt.float32
    BF = mybir.dt.bfloat16
    F16 = mybir.dt.float16
    AF = mybir.ActivationFunctionType

    n_pass = b_loc // w
    assert n_pass * w == b_loc
    chunks = [(c, min(mm_n, w - c)) for c in range(0, w, mm_n)]

    nc = bass.Bass()
    s_ext = nc.dram_tensor("s", [SCENE, b_loc], BF, kind="ExternalInput")
    eps_ext = nc.dram_tensor("eps", [NS, FEAT, b_loc], F32, kind="ExternalInput")
    wx_ext = nc.dram_tensor("wx", [FEAT, G4], BF, kind="ExternalInput")
    wh_ext = nc.dram_tensor("wh", [HID, G4], BF, kind="ExternalInput")
    whd_ext = nc.dram_tensor("whd", [HID, 2 * FEAT], BF, kind="ExternalInput")
    we_ext = nc.dram_tensor("we", [SCENE, FEAT], BF, kind="ExternalInput")
    bias_ext = nc.dram_tensor("bias", [P, 38], F32, kind="ExternalInput")
    oz_ext = nc.dram_tensor("oz", [NS, FEAT, b_loc], F32, kind="ExternalOutput")
    omu_ext = nc.dram_tensor("omu", [NS, FEAT, b_loc], F32, kind="ExternalOutput")
    osg_ext = nc.dram_tensor("osg", [NS, FEAT, b_loc], F32, kind="ExternalOutput")

    with tile.TileContext(nc) as tc:
        with (
            tc.tile_pool(name="wp", bufs=1) as wp,
            tc.tile_pool(name="work", bufs=2) as dp,
            tc.tile_pool(name="psum", bufs=8, space="PSUM") as pp,
        ):
            # DMA order matters: the encoder and slot 0 need only we/wx/bias,
            # so load those before the big (8MB) W_hh to let PE start early.
            we_sb = wp.tile([P, KX, FEAT], BF, tag="we", name="we_sb")
            nc.sync.dma_start(we_sb[:], we_ext.rearrange("(ko p) m -> p ko m", p=P))
            bias_sb = wp.tile([P, 38], F32, tag="bias", name="bias_sb")
            nc.sync.dma_start(bias_sb[:], bias_ext[:])
            # wx/wh/whd are not needed until slot-0 gates / heads(0) /
            # gates(1); their DMAs are emitted after pass 0's s DMA so the
            # encoder's inputs don't queue behind 11MB of weight traffic.
            wx_sb = wp.tile([P, KX, G4], BF, tag="wx", name="wx_sb")
            whd_sb = wp.tile([P, KH, 2 * FEAT], BF, tag="whd", name="whd_sb")
            wh_sb = wp.tile([P, KH, G4], BF, tag="wh", name="wh_sb")

            for p_i in range(n_pass):
                col0 = p_i * w
                c_sb = dp.tile([P, KH, w], F16, tag="c", bufs=1, name="c_sb")

                # encoder: x0 = gelu(We @ S_loc.T + be), feature-major
                s_sb = dp.tile([P, KX, w], BF, tag="s", bufs=1, name="s_sb")
                nc.sync.dma_start(
                    s_sb[:],
                    s_ext.rearrange("(po p) b -> p po b", p=P)[:, :, col0 : col0 + w],
                )
                if p_i == 0:
                    nc.sync.dma_start(
                        wx_sb[:], wx_ext.rearrange("(ko p) m -> p ko m", p=P)
                    )
                    nc.sync.dma_start(
                        whd_sb[:], whd_ext.rearrange("(ko p) m -> p ko m", p=P)
                    )
                    nc.sync.dma_start(
                        wh_sb[:], wh_ext.rearrange("(ko p) m -> p ko m", p=P)
                    )
                x_cur = dp.tile([P, PO, w], BF, tag="x", bufs=2, name="x0_sb")
                for fi in range(PO):
                    pss = [
                        pp.tile([P, cw], F32, tag="ps", name="ps_enc")
                        for (c0, cw) in chunks
                    ]
                    for k in range(KX):
                        for ci, (c0, cw) in enumerate(chunks):
                            nc.tensor.matmul(
                                pss[ci][:],
                                we_sb[:, k, fi * P : (fi + 1) * P],
                                s_sb[:, k, c0 : c0 + cw],
                                start=(k == 0),
                                stop=(k == KX - 1),
                            )
                    for ci, (c0, cw) in enumerate(chunks):
                        nc.scalar.activation(
                            x_cur[:, fi, c0 : c0 + cw],
                            pss[ci][:],
                            AF.Gelu,
                            bias=bias_sb[:, 32 + fi : 33 + fi],
                        )

                h_prev = None
                for t in range(n_slots):
                    eps_sb = dp.tile([P, PO, w], F32, tag="eps", bufs=2, name="eps_sb")
                    nc.sync.dma_start(
                        eps_sb[:],
                        eps_ext[t].rearrange("(po p) b -> p po b", p=P)[
                            :, :, col0 : col0 + w
                        ],
                    )
                    h_new = dp.tile([P, KH, w], BF, tag="h", bufs=2, name="h_sb")
                    # (r_idx, o_gate_tile) whose tanh(c)/h-multiply is
                    # deferred until after the NEXT r-block's gate evictions,
                    # so ScalarE's eviction stream never stalls on the DVE
                    # c-update (that stall starves PSUM recycling and PE).
                    pend = None

                    def flush_pend():
                        nonlocal pend
                        if pend is None:
                            return
                        rp, gop = pend
                        th = dp.tile([P, w], BF, tag="th", bufs=2, name="th_sb")
                        nc.scalar.activation(th[:], c_sb[:, rp], AF.Tanh)
                        nc.vector.tensor_mul(h_new[:, rp], gop[:], th[:])
                        pend = None

                    for r in range(KH):
                        gts = {}
                        # slot 0: c=0, so the forget gate is never used
                        gate_ids = (0, 2, 3) if h_prev is None else (0, 1, 2, 3)
                        for g in gate_ids:
                            m = g * KH + r
                            pss = [
                                pp.tile([P, cw], F32, tag="ps", name="ps_g")
                                for (c0, cw) in chunks
                            ]
                            if h_prev is not None:
                                for k in range(KH):
                                    for ci, (c0, cw) in enumerate(chunks):
                                        nc.tensor.matmul(
                                            pss[ci][:],
                                            wh_sb[:, k, m * P : (m + 1) * P],
                                            h_prev[:, k, c0 : c0 + cw],
                                            start=(k == 0),
                                            stop=False,
                                        )
                                for k in range(KX):
                                    for ci, (c0, cw) in enumerate(chunks):
                                        nc.tensor.matmul(
                                            pss[ci][:],
                                            wx_sb[:, k, m * P : (m + 1) * P],
                                            x_cur[:, k, c0 : c0 + cw],
                                            start=False,
                                            stop=(k == KX - 1),
                                        )
                            else:
                                for k in range(KX):
                                    for ci, (c0, cw) in enumerate(chunks):
                                        nc.tensor.matmul(
                                            pss[ci][:],
                                            wx_sb[:, k, m * P : (m + 1) * P],
                                            x_cur[:, k, c0 : c0 + cw],
                                            start=(k == 0),
                                            stop=(k == KX - 1),
                                        )
                            # o-gate is double-buffered: its consumer (the
                            # deferred h-multiply) runs one r-block late
                            gt = dp.tile(
                                [P, w],
                                BF,
                                tag=f"g{g}",
                                bufs=2 if g == 3 else 1,
                                name=f"g{g}_sb",
                            )
                            func = AF.Tanh if g == 2 else AF.Sigmoid
                            for ci, (c0, cw) in enumerate(chunks):
                                nc.scalar.activation(
                                    gt[:, c0 : c0 + cw],
                                    pss[ci][:],
                                    func,
                                    bias=bias_sb[:, m : m + 1],
                                )
                            gts[g] = gt
                        flush_pend()
                        gi, gf, gg, go = (gts.get(g) for g in range(4))
                        if h_prev is not None:
                            t1 = dp.tile([P, w], BF, tag="t1", bufs=1, name="t1_sb")
                            nc.vector.tensor_mul(t1[:], gi[:], gg[:])
                            t2 = dp.tile([P, w], F32, tag="t2", bufs=2, name="t2_sb")
                            nc.vector.tensor_mul(t2[:], gf[:], c_sb[:, r])
                            nc.vector.tensor_add(c_sb[:, r], t1[:], t2[:])
                        else:
                            nc.vector.tensor_mul(c_sb[:, r], gi[:], gg[:])
                        pend = (r, go)
                    flush_pend()

                    # heads: [mu(256); softplus_pre(256)] = Whd.T @ h.
                    # The sigma half runs first: its ACT chain (Exp -> Ln ->
                    # ze) is the long pole toward the next slot's x, and the
                    # Ln/ze ops overlap the mu head matmuls.
                    mu_sb = dp.tile([P, PO, w], F32, tag="mu", bufs=1, name="mu_sb")
                    sg_sb = dp.tile([P, PO, w], F32, tag="sg", bufs=1, name="sg_sb")
                    x_next = dp.tile([P, PO, w], BF, tag="x", bufs=2, name="x_sb")
                    last_slot = p_i == n_pass - 1 and t == n_slots - 1
                    ze = [None, None]
                    for hm in (PO, PO + 1, 0, 1):  # sigma halves, then mu
                        pss = [
                            pp.tile([P, cw], F32, tag="ps", name="ps_h")
                            for (c0, cw) in chunks
                        ]
                        for k in range(KH):
                            for ci, (c0, cw) in enumerate(chunks):
                                nc.tensor.matmul(
                                    pss[ci][:],
                                    whd_sb[:, k, hm * P : (hm + 1) * P],
                                    h_new[:, k, c0 : c0 + cw],
                                    start=(k == 0),
                                    stop=(k == KH - 1),
                                )
                        if hm >= PO:
                            # softplus(u) = ln(1 + exp(u)); this toolchain has
                            # no softplus ACT table, but exp and ln share one.
                            po = hm - PO
                            for ci, (c0, cw) in enumerate(chunks):
                                nc.scalar.activation(
                                    sg_sb[:, po, c0 : c0 + cw],
                                    pss[ci][:],
                                    AF.Exp,
                                    bias=bias_sb[:, 36 + po : 37 + po],
                                )
                            nc.scalar.activation(
                                sg_sb[:, po], sg_sb[:, po], AF.Ln, bias=1.0
                            )
                            zt = dp.tile([P, w], F32, tag="t2", bufs=2, name="ze_sb")
                            nc.vector.tensor_mul(zt[:], sg_sb[:, po], eps_sb[:, po])
                            ze[po] = zt
                            nc.sync.dma_start(
                                osg_ext[t].rearrange("(po p) b -> p po b", p=P)[
                                    :, po : po + 1, col0 : col0 + w
                                ],
                                sg_sb[:, po : po + 1],
                            )
                        else:
                            po = hm
                            for ci, (c0, cw) in enumerate(chunks):
                                nc.scalar.activation(
                                    mu_sb[:, po, c0 : c0 + cw],
                                    pss[ci][:],
                                    AF.Identity,
                                    bias=bias_sb[:, 34 + po : 35 + po],
                                )
                            # z = mu + sigma*eps, overwriting the eps tile
                            nc.vector.tensor_add(
                                eps_sb[:, po], ze[po][:], mu_sb[:, po]
                            )
                            if not last_slot:
                                nc.vector.tensor_copy(x_next[:, po], eps_sb[:, po])
                            nc.sync.dma_start(
                                oz_ext[t].rearrange("(po p) b -> p po b", p=P)[
                                    :, po : po + 1, col0 : col0 + w
                                ],
                                eps_sb[:, po : po + 1],
                            )
                            nc.sync.dma_start(
                                omu_ext[t].rearrange("(po p) b -> p po b", p=P)[
                                    :, po : po + 1, col0 : col0 + w
                                ],
                                mu_sb[:, po : po + 1],
                            )
                    x_cur = x_next
                    h_prev = h_new
    return nc


_NC_CACHE = {}


def _get_nc(b_loc, w, n_slots):
    key = (b_loc, w, n_slots)
    if key not in _NC_CACHE:
        _NC_CACHE[key] = build(b_loc, w, n_slots)
    return _NC_CACHE[key]


def _prep_shared(We, be, W_ih, W_hh, b_ih, b_hh, W_mu, b_mu, W_sp, b_sp):
    f32 = np.float32
    wx = np.ascontiguousarray(np.asarray(W_ih, f32).T).astype(BF16)
    wh = np.ascontiguousarray(np.asarray(W_hh, f32).T).astype(BF16)
    whd = np.ascontiguousarray(
        np.concatenate(
            [np.asarray(W_mu, f32).T, np.asarray(W_sp, f32).T], axis=1
        )
    ).astype(BF16)
    we = np.ascontiguousarray(np.asarray(We, f32).T).astype(BF16)
    bg = (np.asarray(b_ih, f32) + np.asarray(b_hh, f32)).reshape(MT, P).T
    beT = np.asarray(be, f32).reshape(PO, P).T
    bmuT = np.asarray(b_mu, f32).reshape(PO, P).T
    bspT = np.asarray(b_sp, f32).reshape(PO, P).T
    bias = np.ascontiguousarray(
        np.concatenate([bg, beT, bmuT, bspT], axis=1), dtype=f32
    )
    return {"wx": wx, "wh": wh, "whd": whd, "we": we, "bias": bias}


def _prep_in_maps(S, eps, shared, n_cores=N_CORES, b_loc=B_LOC):
    f32 = np.float32
    S = np.asarray(S, f32)
    eps = np.asarray(eps, f32)
    in_maps = []
    for ci in range(n_cores):
        rows = slice(ci * b_loc, (ci + 1) * b_loc)
        s_t = np.ascontiguousarray(S[rows].T).astype(BF16)
        eps_t = np.ascontiguousarray(eps[:NS, rows, :].transpose(0, 2, 1))
        in_maps.append({"s": s_t, "eps": eps_t, **shared})
    return in_maps


def _run(inputs, trace=False):
    from concourse.bass_utils import run_bass_kernel_spmd

    num_slots = int(inputs.get("num_slots", NS))
    nc = _get_nc(B_LOC, PASS_W, NS)
    shared = _prep_shared(
        inputs["We"], inputs["be"], inputs["W_ih"], inputs["W_hh"],
        inputs["b_ih"], inputs["b_hh"], inputs["W_mu"], inputs["b_mu"],
        inputs["W_sp"], inputs["b_sp"],
    )
    in_maps = _prep_in_maps(inputs["S"], inputs["eps"], shared)
    res = run_bass_kernel_spmd(
        nc, in_maps, core_ids=list(range(N_CORES)), trace=trace
    )
    zs = np.empty((NS, B, FEAT), np.float32)
    mus = np.empty((NS, B, FEAT), np.float32)
    sgs = np.empty((NS, B, FEAT), np.float32)
    for ci in range(N_CORES):
        rows = slice(ci * B_LOC, (ci + 1) * B_LOC)
        zs[:, rows, :] = res.results[ci]["oz"].transpose(0, 2, 1)
        mus[:, rows, :] = res.results[ci]["omu"].transpose(0, 2, 1)
        sgs[:, rows, :] = res.results[ci]["osg"].transpose(0, 2, 1)
    return (zs[:num_slots], mus[:num_slots], sgs[:num_slots]), res.exec_time_ns


def kernel(**inputs):
    out, _ = _run(inputs, trace=False)
    return out



# revision 11
# speedup vs baseline: 1.6303x; 1.6303x over previous
"""Trainium2 Bass kernel for nn_AutoregressivePrior (8-slot LSTM prior).

Strategy: pure data-parallel over batch (16384 rows -> 2048 per NeuronCore),
weights replicated. Feature-major dataflow on chip: every activation lives as
[feature_partition, batch_free] so LSTM matmul chains never transpose.

The recurrent W_hh @ h matmul (~70% of all FLOPs) runs in fp8-e4m3 with
perf_mode=DoubleRow (2 fp8 weights per PE cell -> ~2x MAC rate): contraction
pairs two 128-k-tiles per instruction. h's contribution to the gates is small
relative to x's, so its fp8 quantization error is diluted (~8e-3 final rel
err vs the 2e-2 budget). Everything else (encoder, W_ih @ x, heads) stays
bf16 (PSUM accumulates fp32); bf16 and DoubleRow-fp8 matmuls accumulate into
the same PSUM group. Gate nonlinearities fused with bias on ScalarE straight
out of PSUM, cell state held in fp16 in SBUF.

Per core the 2048-row batch is processed in two sequential passes of 1024
columns so all state fits in SBUF.

Inputs arrive as full-size numpy arrays; outputs are returned full-size
(zs, mus, sigmas) each [num_slots, 16384, 256] fp32, matching the reference.
"""

import sys

if "/opt/trn_rl_repo" not in sys.path:
    sys.path.insert(0, "/opt/trn_rl_repo")

import numpy as np
import ml_dtypes

BF16 = ml_dtypes.bfloat16
FP8E4 = ml_dtypes.float8_e4m3

B = 16384
N_CORES = 8
B_LOC = B // N_CORES  # 2048
SCENE = 256
FEAT = 256
HID = 1024
G4 = 4 * HID  # 4096
NS = 8
P = 128
PO = FEAT // P  # 2
KH = HID // P  # 8
KX = FEAT // P  # 2
MT = G4 // P  # 32
HM = (2 * FEAT) // P  # 4 head m-tiles (mu 2 + softplus 2)

PASS_W = 1024  # batch columns per pass on chip

_PATCHED = False


def _patch_tile_drain():
    """walrus in this toolchain rejects >1 sync-wait on a single instruction;
    split excess waits onto standalone single-wait EventSemaphore instructions
    that run on the same engine immediately before the original instruction."""
    global _PATCHED
    if _PATCHED:
        return
    import bass_rust
    import concourse.tile as tile
    from concourse import mybir
    from concourse.vector_clock import ScopedClock

    MAXW = 1
    _orig_lower = tile.TileContext._lower_ordered_insts

    def _lower_split_waits(self, ordered):
        nc = self.nc
        for bbn, insts in ordered.items():
            out = []
            for inst in insts:
                si = getattr(inst, "sync_info", None)
                if si is not None:
                    waits = list(si.on_wait)
                    if len(waits) > MAXW:
                        imm = [w for w in waits if w.wait_mode == "sem-ge-imm"]
                        other = [w for w in waits if w.wait_mode != "sem-ge-imm"]
                        assert len(other) <= MAXW, (inst.name, waits)
                        keep_n = MAXW - len(other)
                        if keep_n > 0:
                            move = imm[: len(imm) - keep_n]
                            keep = imm[len(imm) - keep_n :]
                        else:
                            move = imm
                            keep = []
                        for wt in move:
                            wi = mybir.InstEventSemaphore(
                                name=nc.get_next_instruction_name(),
                                ins=[],
                                outs=[],
                                engine=inst.engine,
                            )
                            wi.sync_info = bass_rust.SyncInfo(
                                on_wait=[wt], on_update=[]
                            )
                            out.append(wi)
                        si.on_wait = other + keep
                out.append(inst)
            insts[:] = out
        return _orig_lower(self, ordered)

    tile.TileContext._lower_ordered_insts = _lower_split_waits

    def _drain_and_barrier(self, tick_clock, wait_clock):
        nc = self.nc
        drain_inst = nc.sync.drain()
        wait_clock.add_sem_waits(
            drain_inst.ins, ScopedClock({None: tick_clock.global_clock})
        )
        si = drain_inst.ins.sync_info
        if si is not None and len(si.on_wait) > 1:
            waits = list(si.on_wait)
            si.on_wait = waits[:1]
            name2handle = {h.name: h for h in self.sems.allocated().values()}
            for w in waits[1:]:
                assert w.wait_mode == "sem-ge-imm", w
                nc.sync.wait_ge(name2handle[w.ant_name], w.wait_value)
        nc.all_engine_barrier()
        popped = nc._tile_sem_poison_stack.pop()
        assert popped is self._sem_poison
        nc.clear_and_free_semaphores(list(self.sems.allocated().values()))
        nc.all_engine_barrier()

    tile.TileContext._drain_and_barrier = _drain_and_barrier
    _PATCHED = True


def build(b_loc=B_LOC, w=PASS_W, n_slots=NS, mm_n=512):
    _patch_tile_drain()
    import concourse.bass as bass
    import concourse.tile as tile
    from concourse import mybir

    F32 = mybir.dt.float32
    BF = mybir.dt.bfloat16
    F16 = mybir.dt.float16
    F8 = mybir.dt.float8e4
    AF = mybir.ActivationFunctionType
    DR = mybir.MatmulPerfMode.DoubleRow

    n_pass = b_loc // w
    assert n_pass * w == b_loc
    chunks = [(c, min(mm_n, w - c)) for c in range(0, w, mm_n)]

    nc = bass.Bass()
    s_ext = nc.dram_tensor("s", [SCENE, b_loc], BF, kind="ExternalInput")
    eps_ext = nc.dram_tensor("eps", [NS, FEAT, b_loc], F32, kind="ExternalInput")
    wx_ext = nc.dram_tensor("wx", [FEAT, G4], BF, kind="ExternalInput")
    wh_ext = nc.dram_tensor("wh", [HID, G4], F8, kind="ExternalInput")
    whd_ext = nc.dram_tensor("whd", [HID, 2 * FEAT], BF, kind="ExternalInput")
    we_ext = nc.dram_tensor("we", [SCENE, FEAT], BF, kind="ExternalInput")
    bias_ext = nc.dram_tensor("bias", [P, 38], F32, kind="ExternalInput")
    oz_ext = nc.dram_tensor("oz", [NS, FEAT, b_loc], F32, kind="ExternalOutput")
    omu_ext = nc.dram_tensor("omu", [NS, FEAT, b_loc], F32, kind="ExternalOutput")
    osg_ext = nc.dram_tensor("osg", [NS, FEAT, b_loc], F32, kind="ExternalOutput")

    with tile.TileContext(nc) as tc:
        with (
            tc.tile_pool(name="wp", bufs=1) as wp,
            tc.tile_pool(name="work", bufs=2) as dp,
            tc.tile_pool(name="psum", bufs=8, space="PSUM") as pp,
        ):
            # DMA order matters: the encoder and slot 0 need only we/wx/bias,
            # so load those before the big (8MB) W_hh to let PE start early.
            we_sb = wp.tile([P, KX, FEAT], BF, tag="we", name="we_sb")
            nc.sync.dma_start(we_sb[:], we_ext.rearrange("(ko p) m -> p ko m", p=P))
            bias_sb = wp.tile([P, 38], F32, tag="bias", name="bias_sb")
            nc.sync.dma_start(bias_sb[:], bias_ext[:])
            # wx/wh/whd are not needed until slot-0 gates / heads(0) /
            # gates(1); their DMAs are emitted after pass 0's s DMA so the
            # encoder's inputs don't queue behind 11MB of weight traffic.
            wx_sb = wp.tile([P, KX, G4], BF, tag="wx", name="wx_sb")
            whd_sb = wp.tile([P, KH, 2 * FEAT], BF, tag="whd", name="whd_sb")
            wh_sb = wp.tile([P, KH, G4], F8, tag="wh", name="wh_sb")

            for p_i in range(n_pass):
                col0 = p_i * w
                c_sb = dp.tile([P, KH, w], F16, tag="c", bufs=1, name="c_sb")

                # encoder: x0 = gelu(We @ S_loc.T + be), feature-major
                s_sb = dp.tile([P, KX, w], BF, tag="s", bufs=1, name="s_sb")
                nc.sync.dma_start(
                    s_sb[:],
                    s_ext.rearrange("(po p) b -> p po b", p=P)[:, :, col0 : col0 + w],
                )
                if p_i == 0:
                    nc.sync.dma_start(
                        wx_sb[:], wx_ext.rearrange("(ko p) m -> p ko m", p=P)
                    )
                    nc.sync.dma_start(
                        whd_sb[:], whd_ext.rearrange("(ko p) m -> p ko m", p=P)
                    )
                    nc.sync.dma_start(
                        wh_sb[:], wh_ext.rearrange("(ko p) m -> p ko m", p=P)
                    )
                x_cur = dp.tile([P, PO, w], BF, tag="x", bufs=2, name="x0_sb")
                for fi in range(PO):
                    pss = [
                        pp.tile([P, cw], F32, tag="ps", name="ps_enc")
                        for (c0, cw) in chunks
                    ]
                    for k in range(KX):
                        for ci, (c0, cw) in enumerate(chunks):
                            nc.tensor.matmul(
                                pss[ci][:],
                                we_sb[:, k, fi * P : (fi + 1) * P],
                                s_sb[:, k, c0 : c0 + cw],
                                start=(k == 0),
                                stop=(k == KX - 1),
                            )
                    for ci, (c0, cw) in enumerate(chunks):
                        nc.scalar.activation(
                            x_cur[:, fi, c0 : c0 + cw],
                            pss[ci][:],
                            AF.Gelu,
                            bias=bias_sb[:, 32 + fi : 33 + fi],
                        )

                h8_prev = None
                for t in range(n_slots):
                    eps_sb = dp.tile([P, PO, w], F32, tag="eps", bufs=2, name="eps_sb")
                    nc.sync.dma_start(
                        eps_sb[:],
                        eps_ext[t].rearrange("(po p) b -> p po b", p=P)[
                            :, :, col0 : col0 + w
                        ],
                    )
                    h_new = dp.tile([P, KH, w], BF, tag="h", bufs=2, name="h_sb")
                    # fp8 copy of h feeds the next slot's DoubleRow gate
                    # matmuls; the bf16 h feeds this slot's head matmuls.
                    need_h8 = t < n_slots - 1
                    h8_new = (
                        dp.tile([P, KH, w], F8, tag="h8", bufs=2, name="h8_sb")
                        if need_h8
                        else None
                    )
                    # (r_idx, o_gate_tile) whose tanh(c)/h-multiply is
                    # deferred until after the NEXT r-block's gate evictions,
                    # so ScalarE's eviction stream never stalls on the DVE
                    # c-update (that stall starves PSUM recycling and PE).
                    pend = None

                    def flush_pend():
                        nonlocal pend
                        if pend is None:
                            return
                        rp, gop = pend
                        th = dp.tile([P, w], BF, tag="th", bufs=2, name="th_sb")
                        nc.scalar.activation(th[:], c_sb[:, rp], AF.Tanh)
                        nc.vector.tensor_mul(h_new[:, rp], gop[:], th[:])
                        if need_h8:
                            nc.vector.tensor_copy(h8_new[:, rp], h_new[:, rp])
                        pend = None

                    for r in range(KH):
                        gts = {}
                        # slot 0: c=0, so the forget gate is never used
                        gate_ids = (0, 2, 3) if h8_prev is None else (0, 1, 2, 3)
                        for g in gate_ids:
                            m = g * KH + r
                            pss = [
                                pp.tile([P, cw], F32, tag="ps", name="ps_g")
                                for (c0, cw) in chunks
                            ]
                            if h8_prev is not None:
                                for j in range(KH // 2):
                                    for ci, (c0, cw) in enumerate(chunks):
                                        nc.tensor.matmul(
                                            pss[ci][:],
                                            wh_sb[
                                                :, 2 * j : 2 * j + 2,
                                                m * P : (m + 1) * P,
                                            ],
                                            h8_prev[:, 2 * j : 2 * j + 2, c0 : c0 + cw],
                                            start=(j == 0),
                                            stop=False,
                                            perf_mode=DR,
                                        )
                                for k in range(KX):
                                    for ci, (c0, cw) in enumerate(chunks):
                                        nc.tensor.matmul(
                                            pss[ci][:],
                                            wx_sb[:, k, m * P : (m + 1) * P],
                                            x_cur[:, k, c0 : c0 + cw],
                                            start=False,
                                            stop=(k == KX - 1),
                                        )
                            else:
                                for k in range(KX):
                                    for ci, (c0, cw) in enumerate(chunks):
                                        nc.tensor.matmul(
                                            pss[ci][:],
                                            wx_sb[:, k, m * P : (m + 1) * P],
                                            x_cur[:, k, c0 : c0 + cw],
                                            start=(k == 0),
                                            stop=(k == KX - 1),
                                        )
                            # o-gate is double-buffered: its consumer (the
                            # deferred h-multiply) runs one r-block late
                            gt = dp.tile(
                                [P, w],
                                BF,
                                tag=f"g{g}",
                                bufs=2 if g == 3 else 1,
                                name=f"g{g}_sb",
                            )
                            func = AF.Tanh if g == 2 else AF.Sigmoid
                            for ci, (c0, cw) in enumerate(chunks):
                                nc.scalar.activation(
                                    gt[:, c0 : c0 + cw],
                                    pss[ci][:],
                                    func,
                                    bias=bias_sb[:, m : m + 1],
                                )
                            gts[g] = gt
                        flush_pend()
                        gi, gf, gg, go = (gts.get(g) for g in range(4))
                        if h8_prev is not None:
                            t1 = dp.tile([P, w], BF, tag="t1", bufs=1, name="t1_sb")
                            nc.vector.tensor_mul(t1[:], gi[:], gg[:])
                            t2 = dp.tile([P, w], F32, tag="t2", bufs=2, name="t2_sb")
                            nc.vector.tensor_mul(t2[:], gf[:], c_sb[:, r])
                            nc.vector.tensor_add(c_sb[:, r], t1[:], t2[:])
                        else:
                            nc.vector.tensor_mul(c_sb[:, r], gi[:], gg[:])
                        pend = (r, go)
                    flush_pend()

                    # heads: [mu(256); softplus_pre(256)] = Whd.T @ h.
                    # The sigma half runs first: its ACT chain (Exp -> Ln ->
                    # ze) is the long pole toward the next slot's x, and the
                    # Ln/ze ops overlap the mu head matmuls.
                    mu_sb = dp.tile([P, PO, w], F32, tag="mu", bufs=1, name="mu_sb")
                    sg_sb = dp.tile([P, PO, w], F32, tag="sg", bufs=1, name="sg_sb")
                    x_next = dp.tile([P, PO, w], BF, tag="x", bufs=2, name="x_sb")
                    last_slot = p_i == n_pass - 1 and t == n_slots - 1
                    ze = [None, None]
                    for hm in (PO, PO + 1, 0, 1):  # sigma halves, then mu
                        pss = [
                            pp.tile([P, cw], F32, tag="ps", name="ps_h")
                            for (c0, cw) in chunks
                        ]
                        for k in range(KH):
                            for ci, (c0, cw) in enumerate(chunks):
                                nc.tensor.matmul(
                                    pss[ci][:],
                                    whd_sb[:, k, hm * P : (hm + 1) * P],
                                    h_new[:, k, c0 : c0 + cw],
                                    start=(k == 0),
                                    stop=(k == KH - 1),
                                )
                        if hm >= PO:
                            # softplus(u) = ln(1 + exp(u)); this toolchain has
                            # no softplus ACT table, but exp and ln share one.
                            po = hm - PO
                            for ci, (c0, cw) in enumerate(chunks):
                                nc.scalar.activation(
                                    sg_sb[:, po, c0 : c0 + cw],
                                    pss[ci][:],
                                    AF.Exp,
                                    bias=bias_sb[:, 36 + po : 37 + po],
                                )
                            nc.scalar.activation(
                                sg_sb[:, po], sg_sb[:, po], AF.Ln, bias=1.0
                            )
                            zt = dp.tile([P, w], F32, tag="t2", bufs=2, name="ze_sb")
                            nc.vector.tensor_mul(zt[:], sg_sb[:, po], eps_sb[:, po])
                            ze[po] = zt
                            nc.sync.dma_start(
                                osg_ext[t].rearrange("(po p) b -> p po b", p=P)[
                                    :, po : po + 1, col0 : col0 + w
                                ],
                                sg_sb[:, po : po + 1],
                            )
                        else:
                            po = hm
                            for ci, (c0, cw) in enumerate(chunks):
                                nc.scalar.activation(
                                    mu_sb[:, po, c0 : c0 + cw],
                                    pss[ci][:],
                                    AF.Identity,
                                    bias=bias_sb[:, 34 + po : 35 + po],
                                )
                            # z = mu + sigma*eps, overwriting the eps tile
                            nc.vector.tensor_add(
                                eps_sb[:, po], ze[po][:], mu_sb[:, po]
                            )
                            if not last_slot:
                                nc.vector.tensor_copy(x_next[:, po], eps_sb[:, po])
                            nc.sync.dma_start(
                                oz_ext[t].rearrange("(po p) b -> p po b", p=P)[
                                    :, po : po + 1, col0 : col0 + w
                                ],
                                eps_sb[:, po : po + 1],
                            )
                            nc.sync.dma_start(
                                omu_ext[t].rearrange("(po p) b -> p po b", p=P)[
                                    :, po : po + 1, col0 : col0 + w
                                ],
                                mu_sb[:, po : po + 1],
                            )
                    x_cur = x_next
                    h8_prev = h8_new
    return nc


_NC_CACHE = {}


def _get_nc(b_loc, w, n_slots):
    key = (b_loc, w, n_slots)
    if key not in _NC_CACHE:
        _NC_CACHE[key] = build(b_loc, w, n_slots)
    return _NC_CACHE[key]


def _prep_shared(We, be, W_ih, W_hh, b_ih, b_hh, W_mu, b_mu, W_sp, b_sp):
    f32 = np.float32
    wx = np.ascontiguousarray(np.asarray(W_ih, f32).T).astype(BF16)
    wh = np.ascontiguousarray(np.asarray(W_hh, f32).T).astype(FP8E4)
    whd = np.ascontiguousarray(
        np.concatenate(
            [np.asarray(W_mu, f32).T, np.asarray(W_sp, f32).T], axis=1
        )
    ).astype(BF16)
    we = np.ascontiguousarray(np.asarray(We, f32).T).astype(BF16)
    bg = (np.asarray(b_ih, f32) + np.asarray(b_hh, f32)).reshape(MT, P).T
    beT = np.asarray(be, f32).reshape(PO, P).T
    bmuT = np.asarray(b_mu, f32).reshape(PO, P).T
    bspT = np.asarray(b_sp, f32).reshape(PO, P).T
    bias = np.ascontiguousarray(
        np.concatenate([bg, beT, bmuT, bspT], axis=1), dtype=f32
    )
    return {"wx": wx, "wh": wh, "whd": whd, "we": we, "bias": bias}


def _prep_in_maps(S, eps, shared, n_cores=N_CORES, b_loc=B_LOC):
    f32 = np.float32
    S = np.asarray(S, f32)
    eps = np.asarray(eps, f32)
    in_maps = []
    for ci in range(n_cores):
        rows = slice(ci * b_loc, (ci + 1) * b_loc)
        s_t = np.ascontiguousarray(S[rows].T).astype(BF16)
        eps_t = np.ascontiguousarray(eps[:NS, rows, :].transpose(0, 2, 1))
        in_maps.append({"s": s_t, "eps": eps_t, **shared})
    return in_maps


def _run(inputs, trace=False):
    from concourse.bass_utils import run_bass_kernel_spmd

    num_slots = int(inputs.get("num_slots", NS))
    nc = _get_nc(B_LOC, PASS_W, NS)
    shared = _prep_shared(
        inputs["We"], inputs["be"], inputs["W_ih"], inputs["W_hh"],
        inputs["b_ih"], inputs["b_hh"], inputs["W_mu"], inputs["b_mu"],
        inputs["W_sp"], inputs["b_sp"],
    )
    in_maps = _prep_in_maps(inputs["S"], inputs["eps"], shared)
    res = run_bass_kernel_spmd(
        nc, in_maps, core_ids=list(range(N_CORES)), trace=trace
    )
    zs = np.empty((NS, B, FEAT), np.float32)
    mus = np.empty((NS, B, FEAT), np.float32)
    sgs = np.empty((NS, B, FEAT), np.float32)
    for ci in range(N_CORES):
        rows = slice(ci * B_LOC, (ci + 1) * B_LOC)
        zs[:, rows, :] = res.results[ci]["oz"].transpose(0, 2, 1)
        mus[:, rows, :] = res.results[ci]["omu"].transpose(0, 2, 1)
        sgs[:, rows, :] = res.results[ci]["osg"].transpose(0, 2, 1)
    return (zs[:num_slots], mus[:num_slots], sgs[:num_slots]), res.exec_time_ns


def kernel(**inputs):
    out, _ = _run(inputs, trace=False)
    return out



# revision 24
# speedup vs baseline: 1.8014x; 1.1049x over previous
"""Trainium2 Bass kernel for nn_AutoregressivePrior (8-slot LSTM prior).

Strategy: pure data-parallel over batch (16384 rows -> 2048 per NeuronCore),
weights replicated. Feature-major dataflow on chip: every activation lives as
[feature_partition, batch_free] so LSTM matmul chains never transpose.

Most matmul FLOPs run in fp8-e4m3 with perf_mode=DoubleRow (2 fp8 weights
per PE cell -> 2x MAC rate; contraction pairs two 128-k-tiles per
instruction): the recurrent W_hh @ h (~70% of FLOPs; h's contribution to the
gates is small relative to x's, so its quantization error is diluted), the
W_ih @ x parts of the i/f/o gates (their error is compressed by saturating
sigmoids), and the softplus/sigma head. Precision-critical paths stay bf16:
the g-gate's W_ih @ x (feeds tanh content directly into the cell state), the
mu head (graded output with a small norm), and the encoder. Measured final
rel err ~1.2e-2 vs the 2e-2 budget. bf16 and DoubleRow-fp8 matmuls
accumulate into the same PSUM group (fp32). Gate nonlinearities fused with
bias on ScalarE straight out of PSUM, cell state held in fp16 in SBUF.

Per core the 2048-row batch is processed in two sequential passes of 1024
columns so all state fits in SBUF.

Inputs arrive as full-size numpy arrays; outputs are returned full-size
(zs, mus, sigmas) each [num_slots, 16384, 256] fp32, matching the reference.
"""

import sys

if "/opt/trn_rl_repo" not in sys.path:
    sys.path.insert(0, "/opt/trn_rl_repo")

import numpy as np
import ml_dtypes

BF16 = ml_dtypes.bfloat16
FP8E4 = ml_dtypes.float8_e4m3

B = 16384
N_CORES = 8
B_LOC = B // N_CORES  # 2048
SCENE = 256
FEAT = 256
HID = 1024
G4 = 4 * HID  # 4096
NS = 8
P = 128
PO = FEAT // P  # 2
KH = HID // P  # 8
KX = FEAT // P  # 2
MT = G4 // P  # 32
HM = (2 * FEAT) // P  # 4 head m-tiles (mu 2 + softplus 2)

PASS_W = 1024  # batch columns per pass on chip

_PATCHED = False


def _patch_tile_drain():
    """walrus in this toolchain rejects >1 sync-wait on a single instruction;
    split excess waits onto standalone single-wait EventSemaphore instructions
    that run on the same engine immediately before the original instruction."""
    global _PATCHED
    if _PATCHED:
        return
    import bass_rust
    import concourse.tile as tile
    from concourse import mybir
    from concourse.vector_clock import ScopedClock

    MAXW = 1
    _orig_lower = tile.TileContext._lower_ordered_insts

    def _lower_split_waits(self, ordered):
        nc = self.nc
        for bbn, insts in ordered.items():
            out = []
            for inst in insts:
                si = getattr(inst, "sync_info", None)
                if si is not None:
                    waits = list(si.on_wait)
                    if len(waits) > MAXW:
                        imm = [w for w in waits if w.wait_mode == "sem-ge-imm"]
                        other = [w for w in waits if w.wait_mode != "sem-ge-imm"]
                        assert len(other) <= MAXW, (inst.name, waits)
                        keep_n = MAXW - len(other)
                        if keep_n > 0:
                            move = imm[: len(imm) - keep_n]
                            keep = imm[len(imm) - keep_n :]
                        else:
                            move = imm
                            keep = []
                        for wt in move:
                            wi = mybir.InstEventSemaphore(
                                name=nc.get_next_instruction_name(),
                                ins=[],
                                outs=[],
                                engine=inst.engine,
                            )
                            wi.sync_info = bass_rust.SyncInfo(
                                on_wait=[wt], on_update=[]
                            )
                            out.append(wi)
                        si.on_wait = other + keep
                out.append(inst)
            insts[:] = out
        return _orig_lower(self, ordered)

    tile.TileContext._lower_ordered_insts = _lower_split_waits

    def _drain_and_barrier(self, tick_clock, wait_clock):
        nc = self.nc
        drain_inst = nc.sync.drain()
        wait_clock.add_sem_waits(
            drain_inst.ins, ScopedClock({None: tick_clock.global_clock})
        )
        si = drain_inst.ins.sync_info
        if si is not None and len(si.on_wait) > 1:
            waits = list(si.on_wait)
            si.on_wait = waits[:1]
            name2handle = {h.name: h for h in self.sems.allocated().values()}
            for w in waits[1:]:
                assert w.wait_mode == "sem-ge-imm", w
                nc.sync.wait_ge(name2handle[w.ant_name], w.wait_value)
        nc.all_engine_barrier()
        popped = nc._tile_sem_poison_stack.pop()
        assert popped is self._sem_poison
        nc.clear_and_free_semaphores(list(self.sems.allocated().values()))
        nc.all_engine_barrier()

    tile.TileContext._drain_and_barrier = _drain_and_barrier
    _PATCHED = True


def build(b_loc=B_LOC, w=PASS_W, n_slots=NS, mm_n=512):
    _patch_tile_drain()
    import concourse.bass as bass
    import concourse.tile as tile
    from concourse import mybir

    F32 = mybir.dt.float32
    BF = mybir.dt.bfloat16
    F16 = mybir.dt.float16
    F8 = mybir.dt.float8e4
    AF = mybir.ActivationFunctionType
    DR = mybir.MatmulPerfMode.DoubleRow

    n_pass = b_loc // w
    assert n_pass * w == b_loc
    chunks = [(c, min(mm_n, w - c)) for c in range(0, w, mm_n)]

    nc = bass.Bass()
    s_ext = nc.dram_tensor("s", [SCENE, b_loc], BF, kind="ExternalInput")
    eps_ext = nc.dram_tensor("eps", [NS, FEAT, b_loc], F32, kind="ExternalInput")
    wx_ext = nc.dram_tensor("wx", [FEAT, G4], BF, kind="ExternalInput")
    wx8_ext = nc.dram_tensor("wx8", [FEAT, G4], F8, kind="ExternalInput")
    wh_ext = nc.dram_tensor("wh", [HID, G4], F8, kind="ExternalInput")
    whd_ext = nc.dram_tensor("whd", [HID, FEAT], BF, kind="ExternalInput")
    wsp8_ext = nc.dram_tensor("wsp8", [HID, FEAT], F8, kind="ExternalInput")
    we_ext = nc.dram_tensor("we", [SCENE, FEAT], BF, kind="ExternalInput")
    bias_ext = nc.dram_tensor("bias", [P, 38], F32, kind="ExternalInput")
    oz_ext = nc.dram_tensor("oz", [NS, FEAT, b_loc], F32, kind="ExternalOutput")
    omu_ext = nc.dram_tensor("omu", [NS, FEAT, b_loc], F32, kind="ExternalOutput")
    osg_ext = nc.dram_tensor("osg", [NS, FEAT, b_loc], F32, kind="ExternalOutput")

    with tile.TileContext(nc) as tc:
        with (
            tc.tile_pool(name="wp", bufs=1) as wp,
            tc.tile_pool(name="work", bufs=2) as dp,
            tc.tile_pool(name="psum", bufs=8, space="PSUM") as pp,
        ):
            # DMA order matters: the encoder and slot 0 need only we/wx/bias,
            # so load those before the big (8MB) W_hh to let PE start early.
            we_sb = wp.tile([P, KX, FEAT], BF, tag="we", name="we_sb")
            nc.sync.dma_start(we_sb[:], we_ext.rearrange("(ko p) m -> p ko m", p=P))
            bias_sb = wp.tile([P, 38], F32, tag="bias", name="bias_sb")
            nc.sync.dma_start(bias_sb[:], bias_ext[:])
            # wx/wh/whd are not needed until slot-0 gates / heads(0) /
            # gates(1); their DMAs are emitted after pass 0's s DMA so the
            # encoder's inputs don't queue behind 11MB of weight traffic.
            wx_sb = wp.tile([P, KX, G4], BF, tag="wx", name="wx_sb")
            wx8_sb = wp.tile([P, KX, G4], F8, tag="wx8", name="wx8_sb")
            whd_sb = wp.tile([P, KH, FEAT], BF, tag="whd", name="whd_sb")
            wsp8_sb = wp.tile([P, KH, FEAT], F8, tag="wsp8", name="wsp8_sb")
            wh_sb = wp.tile([P, KH, G4], F8, tag="wh", name="wh_sb")

            for p_i in range(n_pass):
                col0 = p_i * w
                c_sb = dp.tile([P, KH, w], F16, tag="c", bufs=1, name="c_sb")

                # encoder: x0 = gelu(We @ S_loc.T + be), feature-major
                s_sb = dp.tile([P, KX, w], BF, tag="s", bufs=1, name="s_sb")
                nc.sync.dma_start(
                    s_sb[:],
                    s_ext.rearrange("(po p) b -> p po b", p=P)[:, :, col0 : col0 + w],
                )
                if p_i == 0:
                    nc.sync.dma_start(
                        wx_sb[:], wx_ext.rearrange("(ko p) m -> p ko m", p=P)
                    )
                    nc.sync.dma_start(
                        wx8_sb[:], wx8_ext.rearrange("(ko p) m -> p ko m", p=P)
                    )
                    nc.sync.dma_start(
                        whd_sb[:], whd_ext.rearrange("(ko p) m -> p ko m", p=P)
                    )
                    nc.sync.dma_start(
                        wsp8_sb[:], wsp8_ext.rearrange("(ko p) m -> p ko m", p=P)
                    )
                    nc.sync.dma_start(
                        wh_sb[:], wh_ext.rearrange("(ko p) m -> p ko m", p=P)
                    )
                x_cur = dp.tile([P, PO, w], BF, tag="x", bufs=2, name="x0_sb")
                for fi in range(PO):
                    pss = [
                        pp.tile([P, cw], F32, tag="ps", name="ps_enc")
                        for (c0, cw) in chunks
                    ]
                    for k in range(KX):
                        for ci, (c0, cw) in enumerate(chunks):
                            nc.tensor.matmul(
                                pss[ci][:],
                                we_sb[:, k, fi * P : (fi + 1) * P],
                                s_sb[:, k, c0 : c0 + cw],
                                start=(k == 0),
                                stop=(k == KX - 1),
                            )
                    for ci, (c0, cw) in enumerate(chunks):
                        nc.scalar.activation(
                            x_cur[:, fi, c0 : c0 + cw],
                            pss[ci][:],
                            AF.Gelu,
                            bias=bias_sb[:, 32 + fi : 33 + fi],
                        )

                h8_prev = None
                x8_cur = None
                for t in range(n_slots):
                    eps_sb = dp.tile([P, PO, w], F32, tag="eps", bufs=2, name="eps_sb")
                    nc.sync.dma_start(
                        eps_sb[:],
                        eps_ext[t].rearrange("(po p) b -> p po b", p=P)[
                            :, :, col0 : col0 + w
                        ],
                    )
                    h_new = dp.tile([P, KH, w], BF, tag="h", bufs=2, name="h_sb")
                    # fp8 copy of h feeds this slot's DoubleRow sigma-head
                    # matmuls and the next slot's DoubleRow gate matmuls; the
                    # bf16 h feeds this slot's mu-head matmuls.
                    h8_new = dp.tile([P, KH, w], F8, tag="h8", bufs=2, name="h8_sb")
                    # (r_idx, o_gate_tile) whose tanh(c)/h-multiply is
                    # deferred until after the NEXT r-block's gate evictions,
                    # so ScalarE's eviction stream never stalls on the DVE
                    # c-update (that stall starves PSUM recycling and PE).
                    pend = None

                    def flush_pend():
                        nonlocal pend
                        if pend is None:
                            return
                        rp, gop = pend
                        th = dp.tile([P, w], BF, tag="th", bufs=2, name="th_sb")
                        nc.scalar.activation(th[:], c_sb[:, rp], AF.Tanh)
                        nc.vector.tensor_mul(h_new[:, rp], gop[:], th[:])
                        nc.vector.tensor_copy(h8_new[:, rp], h_new[:, rp])
                        pend = None

                    for r in range(KH):
                        gts = {}
                        # slot 0: c=0, so the forget gate is never used
                        gate_ids = (0, 2, 3) if h8_prev is None else (0, 1, 2, 3)
                        for g in gate_ids:
                            m = g * KH + r
                            pss = [
                                pp.tile([P, cw], F32, tag="ps", name="ps_g")
                                for (c0, cw) in chunks
                            ]
                            if h8_prev is not None:
                                for j in range(KH // 2):
                                    for ci, (c0, cw) in enumerate(chunks):
                                        nc.tensor.matmul(
                                            pss[ci][:],
                                            wh_sb[
                                                :, 2 * j : 2 * j + 2,
                                                m * P : (m + 1) * P,
                                            ],
                                            h8_prev[:, 2 * j : 2 * j + 2, c0 : c0 + cw],
                                            start=(j == 0),
                                            stop=False,
                                            perf_mode=DR,
                                        )
                                if g != 2:
                                    # i/f/o x-part: one DoubleRow fp8 matmul
                                    # pairing both 128-k-tiles of x
                                    for ci, (c0, cw) in enumerate(chunks):
                                        nc.tensor.matmul(
                                            pss[ci][:],
                                            wx8_sb[:, 0:KX, m * P : (m + 1) * P],
                                            x8_cur[:, 0:KX, c0 : c0 + cw],
                                            start=False,
                                            stop=True,
                                            perf_mode=DR,
                                        )
                                else:
                                    # g-gate x-part feeds tanh content
                                    # directly -> keep bf16
                                    for k in range(KX):
                                        for ci, (c0, cw) in enumerate(chunks):
                                            nc.tensor.matmul(
                                                pss[ci][:],
                                                wx_sb[:, k, m * P : (m + 1) * P],
                                                x_cur[:, k, c0 : c0 + cw],
                                                start=False,
                                                stop=(k == KX - 1),
                                            )
                            else:
                                for k in range(KX):
                                    for ci, (c0, cw) in enumerate(chunks):
                                        nc.tensor.matmul(
                                            pss[ci][:],
                                            wx_sb[:, k, m * P : (m + 1) * P],
                                            x_cur[:, k, c0 : c0 + cw],
                                            start=(k == 0),
                                            stop=(k == KX - 1),
                                        )
                            # o-gate is double-buffered: its consumer (the
                            # deferred h-multiply) runs one r-block late
                            gt = dp.tile(
                                [P, w],
                                BF,
                                tag=f"g{g}",
                                bufs=2 if g == 3 else 1,
                                name=f"g{g}_sb",
                            )
                            func = AF.Tanh if g == 2 else AF.Sigmoid
                            for ci, (c0, cw) in enumerate(chunks):
                                nc.scalar.activation(
                                    gt[:, c0 : c0 + cw],
                                    pss[ci][:],
                                    func,
                                    bias=bias_sb[:, m : m + 1],
                                )
                            gts[g] = gt
                        flush_pend()
                        gi, gf, gg, go = (gts.get(g) for g in range(4))
                        if h8_prev is not None:
                            t1 = dp.tile([P, w], BF, tag="t1", bufs=1, name="t1_sb")
                            nc.vector.tensor_mul(t1[:], gi[:], gg[:])
                            t2 = dp.tile([P, w], F32, tag="t2", bufs=2, name="t2_sb")
                            nc.vector.tensor_mul(t2[:], gf[:], c_sb[:, r])
                            nc.vector.tensor_add(c_sb[:, r], t1[:], t2[:])
                        else:
                            nc.vector.tensor_mul(c_sb[:, r], gi[:], gg[:])
                        pend = (r, go)
                    flush_pend()

                    # heads: [mu(256); softplus_pre(256)] = Whd.T @ h.
                    # The sigma half runs first: its ACT chain (Exp -> Ln ->
                    # ze) is the long pole toward the next slot's x, and the
                    # Ln/ze ops overlap the mu head matmuls.
                    mu_sb = dp.tile([P, PO, w], F32, tag="mu", bufs=1, name="mu_sb")
                    sg_sb = dp.tile([P, PO, w], F32, tag="sg", bufs=1, name="sg_sb")
                    x_next = dp.tile([P, PO, w], BF, tag="x", bufs=2, name="x_sb")
                    x8_next = dp.tile([P, PO, w], F8, tag="x8", bufs=2, name="x8_sb")
                    last_slot = p_i == n_pass - 1 and t == n_slots - 1
                    ze = [None, None]
                    for hm in (PO, PO + 1, 0, 1):  # sigma halves, then mu
                        pss = [
                            pp.tile([P, cw], F32, tag="ps", name="ps_h")
                            for (c0, cw) in chunks
                        ]
                        if hm >= PO:
                            # sigma head: fp8 DoubleRow over h8
                            po8 = hm - PO
                            for j in range(KH // 2):
                                for ci, (c0, cw) in enumerate(chunks):
                                    nc.tensor.matmul(
                                        pss[ci][:],
                                        wsp8_sb[
                                            :, 2 * j : 2 * j + 2,
                                            po8 * P : (po8 + 1) * P,
                                        ],
                                        h8_new[:, 2 * j : 2 * j + 2, c0 : c0 + cw],
                                        start=(j == 0),
                                        stop=(j == KH // 2 - 1),
                                        perf_mode=DR,
                                    )
                        else:
                            # mu head: graded output with small norm -> bf16
                            for k in range(KH):
                                for ci, (c0, cw) in enumerate(chunks):
                                    nc.tensor.matmul(
                                        pss[ci][:],
                                        whd_sb[:, k, hm * P : (hm + 1) * P],
                                        h_new[:, k, c0 : c0 + cw],
                                        start=(k == 0),
                                        stop=(k == KH - 1),
                                    )
                        if hm >= PO:
                            # softplus(u) = ln(1 + exp(u)); this toolchain has
                            # no softplus ACT table, but exp and ln share one.
                            po = hm - PO
                            for ci, (c0, cw) in enumerate(chunks):
                                nc.scalar.activation(
                                    sg_sb[:, po, c0 : c0 + cw],
                                    pss[ci][:],
                                    AF.Exp,
                                    bias=bias_sb[:, 36 + po : 37 + po],
                                )
                            nc.scalar.activation(
                                sg_sb[:, po], sg_sb[:, po], AF.Ln, bias=1.0
                            )
                            zt = dp.tile([P, w], F32, tag="t2", bufs=2, name="ze_sb")
                            nc.vector.tensor_mul(zt[:], sg_sb[:, po], eps_sb[:, po])
                            ze[po] = zt
                            nc.sync.dma_start(
                                osg_ext[t].rearrange("(po p) b -> p po b", p=P)[
                                    :, po : po + 1, col0 : col0 + w
                                ],
                                sg_sb[:, po : po + 1],
                            )
                        else:
                            po = hm
                            for ci, (c0, cw) in enumerate(chunks):
                                nc.scalar.activation(
                                    mu_sb[:, po, c0 : c0 + cw],
                                    pss[ci][:],
                                    AF.Identity,
                                    bias=bias_sb[:, 34 + po : 35 + po],
                                )
                            # z = mu + sigma*eps, overwriting the eps tile
                            nc.vector.tensor_add(
                                eps_sb[:, po], ze[po][:], mu_sb[:, po]
                            )
                            if not last_slot:
                                nc.vector.tensor_copy(x_next[:, po], eps_sb[:, po])
                                nc.vector.tensor_copy(x8_next[:, po], eps_sb[:, po])
                            nc.sync.dma_start(
                                oz_ext[t].rearrange("(po p) b -> p po b", p=P)[
                                    :, po : po + 1, col0 : col0 + w
                                ],
                                eps_sb[:, po : po + 1],
                            )
                            nc.sync.dma_start(
                                omu_ext[t].rearrange("(po p) b -> p po b", p=P)[
                                    :, po : po + 1, col0 : col0 + w
                                ],
                                mu_sb[:, po : po + 1],
                            )
                    x_cur = x_next
                    x8_cur = x8_next
                    h8_prev = h8_new
    return nc


_NC_CACHE = {}


def _get_nc(b_loc, w, n_slots):
    key = (b_loc, w, n_slots)
    if key not in _NC_CACHE:
        _NC_CACHE[key] = build(b_loc, w, n_slots)
    return _NC_CACHE[key]


def _prep_shared(We, be, W_ih, W_hh, b_ih, b_hh, W_mu, b_mu, W_sp, b_sp):
    f32 = np.float32
    wx_t = np.ascontiguousarray(np.asarray(W_ih, f32).T)
    wx = wx_t.astype(BF16)
    wx8 = wx_t.astype(FP8E4)
    wh = np.ascontiguousarray(np.asarray(W_hh, f32).T).astype(FP8E4)
    whd = np.ascontiguousarray(np.asarray(W_mu, f32).T).astype(BF16)
    wsp8 = np.ascontiguousarray(np.asarray(W_sp, f32).T).astype(FP8E4)
    we = np.ascontiguousarray(np.asarray(We, f32).T).astype(BF16)
    bg = (np.asarray(b_ih, f32) + np.asarray(b_hh, f32)).reshape(MT, P).T
    beT = np.asarray(be, f32).reshape(PO, P).T
    bmuT = np.asarray(b_mu, f32).reshape(PO, P).T
    bspT = np.asarray(b_sp, f32).reshape(PO, P).T
    bias = np.ascontiguousarray(
        np.concatenate([bg, beT, bmuT, bspT], axis=1), dtype=f32
    )
    return {
        "wx": wx, "wx8": wx8, "wh": wh, "whd": whd, "wsp8": wsp8,
        "we": we, "bias": bias,
    }


def _prep_in_maps(S, eps, shared, n_cores=N_CORES, b_loc=B_LOC):
    f32 = np.float32
    S = np.asarray(S, f32)
    eps = np.asarray(eps, f32)
    in_maps = []
    for ci in range(n_cores):
        rows = slice(ci * b_loc, (ci + 1) * b_loc)
        s_t = np.ascontiguousarray(S[rows].T).astype(BF16)
        eps_t = np.ascontiguousarray(eps[:NS, rows, :].transpose(0, 2, 1))
        in_maps.append({"s": s_t, "eps": eps_t, **shared})
    return in_maps


def _run(inputs, trace=False):
    from concourse.bass_utils import run_bass_kernel_spmd

    num_slots = int(inputs.get("num_slots", NS))
    nc = _get_nc(B_LOC, PASS_W, NS)
    shared = _prep_shared(
        inputs["We"], inputs["be"], inputs["W_ih"], inputs["W_hh"],
        inputs["b_ih"], inputs["b_hh"], inputs["W_mu"], inputs["b_mu"],
        inputs["W_sp"], inputs["b_sp"],
    )
    in_maps = _prep_in_maps(inputs["S"], inputs["eps"], shared)
    res = run_bass_kernel_spmd(
        nc, in_maps, core_ids=list(range(N_CORES)), trace=trace
    )
    zs = np.empty((NS, B, FEAT), np.float32)
    mus = np.empty((NS, B, FEAT), np.float32)
    sgs = np.empty((NS, B, FEAT), np.float32)
    for ci in range(N_CORES):
        rows = slice(ci * B_LOC, (ci + 1) * B_LOC)
        zs[:, rows, :] = res.results[ci]["oz"].transpose(0, 2, 1)
        mus[:, rows, :] = res.results[ci]["omu"].transpose(0, 2, 1)
        sgs[:, rows, :] = res.results[ci]["osg"].transpose(0, 2, 1)
    return (zs[:num_slots], mus[:num_slots], sgs[:num_slots]), res.exec_time_ns


def kernel(**inputs):
    out, _ = _run(inputs, trace=False)
    return out



# revision 29
# speedup vs baseline: 1.8380x; 1.0203x over previous
"""Trainium2 Bass kernel for nn_AutoregressivePrior (8-slot LSTM prior).

Strategy: pure data-parallel over batch (16384 rows -> 2048 per NeuronCore),
weights replicated. Feature-major dataflow on chip: every activation lives as
[feature_partition, batch_free] so LSTM matmul chains never transpose.

Most matmul FLOPs run in fp8-e4m3 with perf_mode=DoubleRow (2 fp8 weights
per PE cell -> 2x MAC rate; contraction pairs two 128-k-tiles per
instruction): the recurrent W_hh @ h (~70% of FLOPs; h's contribution to the
gates is small relative to x's, so its quantization error is diluted), the
W_ih @ x parts of the i/f/o gates (their error is compressed by saturating
sigmoids), and the softplus/sigma head. Precision-critical paths stay bf16:
the g-gate's W_ih @ x (feeds tanh content directly into the cell state), the
mu head (graded output with a small norm), and the encoder. Measured final
rel err ~1.2e-2 vs the 2e-2 budget. bf16 and DoubleRow-fp8 matmuls
accumulate into the same PSUM group (fp32). Gate nonlinearities fused with
bias on ScalarE straight out of PSUM, cell state held in fp16 in SBUF.

Per core the 2048-row batch is processed in two sequential passes of 1024
columns so all state fits in SBUF.

Inputs arrive as full-size numpy arrays; outputs are returned full-size
(zs, mus, sigmas) each [num_slots, 16384, 256] fp32, matching the reference.
"""

import sys

if "/opt/trn_rl_repo" not in sys.path:
    sys.path.insert(0, "/opt/trn_rl_repo")

import numpy as np
import ml_dtypes

BF16 = ml_dtypes.bfloat16
FP8E4 = ml_dtypes.float8_e4m3

B = 16384
N_CORES = 8
B_LOC = B // N_CORES  # 2048
SCENE = 256
FEAT = 256
HID = 1024
G4 = 4 * HID  # 4096
NS = 8
P = 128
PO = FEAT // P  # 2
KH = HID // P  # 8
KX = FEAT // P  # 2
MT = G4 // P  # 32
HM = (2 * FEAT) // P  # 4 head m-tiles (mu 2 + softplus 2)

PASS_W = 1024  # batch columns per pass on chip

_PATCHED = False


def _patch_tile_drain():
    """walrus in this toolchain rejects >1 sync-wait on a single instruction;
    split excess waits onto standalone single-wait EventSemaphore instructions
    that run on the same engine immediately before the original instruction."""
    global _PATCHED
    if _PATCHED:
        return
    import bass_rust
    import concourse.tile as tile
    from concourse import mybir
    from concourse.vector_clock import ScopedClock

    MAXW = 1
    _orig_lower = tile.TileContext._lower_ordered_insts

    def _lower_split_waits(self, ordered):
        nc = self.nc
        for bbn, insts in ordered.items():
            out = []
            for inst in insts:
                si = getattr(inst, "sync_info", None)
                if si is not None:
                    waits = list(si.on_wait)
                    if len(waits) > MAXW:
                        imm = [w for w in waits if w.wait_mode == "sem-ge-imm"]
                        other = [w for w in waits if w.wait_mode != "sem-ge-imm"]
                        assert len(other) <= MAXW, (inst.name, waits)
                        keep_n = MAXW - len(other)
                        if keep_n > 0:
                            move = imm[: len(imm) - keep_n]
                            keep = imm[len(imm) - keep_n :]
                        else:
                            move = imm
                            keep = []
                        for wt in move:
                            wi = mybir.InstEventSemaphore(
                                name=nc.get_next_instruction_name(),
                                ins=[],
                                outs=[],
                                engine=inst.engine,
                            )
                            wi.sync_info = bass_rust.SyncInfo(
                                on_wait=[wt], on_update=[]
                            )
                            out.append(wi)
                        si.on_wait = other + keep
                out.append(inst)
            insts[:] = out
        return _orig_lower(self, ordered)

    tile.TileContext._lower_ordered_insts = _lower_split_waits

    def _drain_and_barrier(self, tick_clock, wait_clock):
        nc = self.nc
        drain_inst = nc.sync.drain()
        wait_clock.add_sem_waits(
            drain_inst.ins, ScopedClock({None: tick_clock.global_clock})
        )
        si = drain_inst.ins.sync_info
        if si is not None and len(si.on_wait) > 1:
            waits = list(si.on_wait)
            si.on_wait = waits[:1]
            name2handle = {h.name: h for h in self.sems.allocated().values()}
            for w in waits[1:]:
                assert w.wait_mode == "sem-ge-imm", w
                nc.sync.wait_ge(name2handle[w.ant_name], w.wait_value)
        nc.all_engine_barrier()
        popped = nc._tile_sem_poison_stack.pop()
        assert popped is self._sem_poison
        nc.clear_and_free_semaphores(list(self.sems.allocated().values()))
        nc.all_engine_barrier()

    tile.TileContext._drain_and_barrier = _drain_and_barrier
    _PATCHED = True


def build(b_loc=B_LOC, w=PASS_W, n_slots=NS, mm_n=512):
    _patch_tile_drain()
    import concourse.bass as bass
    import concourse.tile as tile
    from concourse import mybir

    F32 = mybir.dt.float32
    BF = mybir.dt.bfloat16
    F16 = mybir.dt.float16
    F8 = mybir.dt.float8e4
    AF = mybir.ActivationFunctionType
    DR = mybir.MatmulPerfMode.DoubleRow

    n_pass = b_loc // w
    assert n_pass * w == b_loc
    chunks = [(c, min(mm_n, w - c)) for c in range(0, w, mm_n)]

    nc = bass.Bass()
    s_ext = nc.dram_tensor("s", [SCENE, b_loc], BF, kind="ExternalInput")
    eps_ext = nc.dram_tensor("eps", [NS, FEAT, b_loc], F32, kind="ExternalInput")
    wx_ext = nc.dram_tensor("wx", [FEAT, G4], BF, kind="ExternalInput")
    wx8_ext = nc.dram_tensor("wx8", [FEAT, G4], F8, kind="ExternalInput")
    wh_ext = nc.dram_tensor("wh", [HID, G4], F8, kind="ExternalInput")
    whd_ext = nc.dram_tensor("whd", [HID, FEAT], BF, kind="ExternalInput")
    wsp8_ext = nc.dram_tensor("wsp8", [HID, FEAT], F8, kind="ExternalInput")
    we_ext = nc.dram_tensor("we", [SCENE, FEAT], BF, kind="ExternalInput")
    bias_ext = nc.dram_tensor("bias", [P, 38], F32, kind="ExternalInput")
    oz_ext = nc.dram_tensor("oz", [NS, FEAT, b_loc], F32, kind="ExternalOutput")
    omu_ext = nc.dram_tensor("omu", [NS, FEAT, b_loc], F32, kind="ExternalOutput")
    osg_ext = nc.dram_tensor("osg", [NS, FEAT, b_loc], F32, kind="ExternalOutput")

    with tile.TileContext(nc) as tc:
        with (
            tc.tile_pool(name="wp", bufs=1) as wp,
            tc.tile_pool(name="work", bufs=2) as dp,
            tc.tile_pool(name="psum", bufs=4, space="PSUM") as pp,
        ):
            # DMA order matters: the encoder and slot 0 need only we/wx/bias,
            # so load those before the big (8MB) W_hh to let PE start early.
            we_sb = wp.tile([P, KX, FEAT], BF, tag="we", name="we_sb")
            nc.sync.dma_start(we_sb[:], we_ext.rearrange("(ko p) m -> p ko m", p=P))
            bias_sb = wp.tile([P, 38], F32, tag="bias", name="bias_sb")
            nc.sync.dma_start(bias_sb[:], bias_ext[:])
            # wx/wh/whd are not needed until slot-0 gates / heads(0) /
            # gates(1); their DMAs are emitted after pass 0's s DMA so the
            # encoder's inputs don't queue behind 11MB of weight traffic.
            wx_sb = wp.tile([P, KX, G4], BF, tag="wx", name="wx_sb")
            wx8_sb = wp.tile([P, KX, G4], F8, tag="wx8", name="wx8_sb")
            whd_sb = wp.tile([P, KH, FEAT], BF, tag="whd", name="whd_sb")
            wsp8_sb = wp.tile([P, KH, FEAT], F8, tag="wsp8", name="wsp8_sb")
            wh_sb = wp.tile([P, KH, G4], F8, tag="wh", name="wh_sb")

            # Both passes' S halves are prefetched up front so pass 1's
            # encoder never queues behind pass 0's output DMAs.
            s_sbs = []
            for p_i in range(n_pass):
                s_sb = dp.tile([P, KX, w], BF, tag="s", bufs=n_pass, name="s_sb")
                nc.sync.dma_start(
                    s_sb[:],
                    s_ext.rearrange("(po p) b -> p po b", p=P)[
                        :, :, p_i * w : (p_i + 1) * w
                    ],
                )
                s_sbs.append(s_sb)
            # the f-gate's bf16 x-part is never used (slot 0 skips f, later
            # slots run it in fp8), so quarter m8..15 of wx is never loaded
            for gq in (0, 2, 3):
                nc.sync.dma_start(
                    wx_sb[:, :, gq * HID : (gq + 1) * HID],
                    wx_ext.rearrange("(ko p) m -> p ko m", p=P)[
                        :, :, gq * HID : (gq + 1) * HID
                    ],
                )
            nc.sync.dma_start(wx8_sb[:], wx8_ext.rearrange("(ko p) m -> p ko m", p=P))
            nc.sync.dma_start(whd_sb[:], whd_ext.rearrange("(ko p) m -> p ko m", p=P))
            nc.sync.dma_start(
                wsp8_sb[:], wsp8_ext.rearrange("(ko p) m -> p ko m", p=P)
            )
            nc.sync.dma_start(wh_sb[:], wh_ext.rearrange("(ko p) m -> p ko m", p=P))

            for p_i in range(n_pass):
                col0 = p_i * w
                c_sb = dp.tile([P, KH, w], F16, tag="c", bufs=1, name="c_sb")
                s_sb = s_sbs[p_i]

                # encoder: x0 = gelu(We @ S_loc.T + be), feature-major
                x_cur = dp.tile([P, PO, w], BF, tag="x", bufs=2, name="x0_sb")
                for fi in range(PO):
                    ps = pp.tile([P, w], F32, tag="ps", name="ps_enc")
                    for k in range(KX):
                        for ci, (c0, cw) in enumerate(chunks):
                            nc.tensor.matmul(
                                ps[:, c0 : c0 + cw],
                                we_sb[:, k, fi * P : (fi + 1) * P],
                                s_sb[:, k, c0 : c0 + cw],
                                start=(k == 0),
                                stop=(k == KX - 1),
                            )
                    nc.scalar.activation(
                        x_cur[:, fi],
                        ps[:],
                        AF.Gelu,
                        bias=bias_sb[:, 32 + fi : 33 + fi],
                    )

                h8_prev = None
                x8_cur = None
                for t in range(n_slots):
                    eps_sb = dp.tile([P, PO, w], F32, tag="eps", bufs=2, name="eps_sb")
                    nc.sync.dma_start(
                        eps_sb[:],
                        eps_ext[t].rearrange("(po p) b -> p po b", p=P)[
                            :, :, col0 : col0 + w
                        ],
                    )
                    h_new = dp.tile([P, KH, w], BF, tag="h", bufs=2, name="h_sb")
                    # fp8 copy of h feeds this slot's DoubleRow sigma-head
                    # matmuls and the next slot's DoubleRow gate matmuls; the
                    # bf16 h feeds this slot's mu-head matmuls.
                    h8_new = dp.tile([P, KH, w], F8, tag="h8", bufs=2, name="h8_sb")
                    # (r_idx, o_gate_tile) whose tanh(c)/h-multiply is
                    # deferred until after the NEXT r-block's gate evictions,
                    # so ScalarE's eviction stream never stalls on the DVE
                    # c-update (that stall starves PSUM recycling and PE).
                    pend = None

                    def flush_pend():
                        nonlocal pend
                        if pend is None:
                            return
                        rp, gop = pend
                        th = dp.tile([P, w], BF, tag="th", bufs=2, name="th_sb")
                        nc.scalar.activation(th[:], c_sb[:, rp], AF.Tanh)
                        nc.vector.tensor_mul(h_new[:, rp], gop[:], th[:])
                        nc.vector.tensor_copy(h8_new[:, rp], h_new[:, rp])
                        pend = None

                    for r in range(KH):
                        gts = {}
                        # slot 0: c=0, so the forget gate is never used
                        gate_ids = (0, 2, 3) if h8_prev is None else (0, 1, 2, 3)
                        for g in gate_ids:
                            m = g * KH + r
                            ps = pp.tile([P, w], F32, tag="ps", name="ps_g")
                            if h8_prev is not None:
                                for j in range(KH // 2):
                                    for ci, (c0, cw) in enumerate(chunks):
                                        nc.tensor.matmul(
                                            ps[:, c0 : c0 + cw],
                                            wh_sb[
                                                :, 2 * j : 2 * j + 2,
                                                m * P : (m + 1) * P,
                                            ],
                                            h8_prev[:, 2 * j : 2 * j + 2, c0 : c0 + cw],
                                            start=(j == 0),
                                            stop=False,
                                            perf_mode=DR,
                                        )
                                if g != 2:
                                    # i/f/o x-part: one DoubleRow fp8 matmul
                                    # pairing both 128-k-tiles of x
                                    for ci, (c0, cw) in enumerate(chunks):
                                        nc.tensor.matmul(
                                            ps[:, c0 : c0 + cw],
                                            wx8_sb[:, 0:KX, m * P : (m + 1) * P],
                                            x8_cur[:, 0:KX, c0 : c0 + cw],
                                            start=False,
                                            stop=True,
                                            perf_mode=DR,
                                        )
                                else:
                                    # g-gate x-part feeds tanh content
                                    # directly -> keep bf16
                                    for k in range(KX):
                                        for ci, (c0, cw) in enumerate(chunks):
                                            nc.tensor.matmul(
                                                ps[:, c0 : c0 + cw],
                                                wx_sb[:, k, m * P : (m + 1) * P],
                                                x_cur[:, k, c0 : c0 + cw],
                                                start=False,
                                                stop=(k == KX - 1),
                                            )
                            else:
                                for k in range(KX):
                                    for ci, (c0, cw) in enumerate(chunks):
                                        nc.tensor.matmul(
                                            ps[:, c0 : c0 + cw],
                                            wx_sb[:, k, m * P : (m + 1) * P],
                                            x_cur[:, k, c0 : c0 + cw],
                                            start=(k == 0),
                                            stop=(k == KX - 1),
                                        )
                            # o-gate is double-buffered: its consumer (the
                            # deferred h-multiply) runs one r-block late
                            gt = dp.tile(
                                [P, w],
                                BF,
                                tag=f"g{g}",
                                bufs=2 if g == 3 else 1,
                                name=f"g{g}_sb",
                            )
                            func = AF.Tanh if g == 2 else AF.Sigmoid
                            nc.scalar.activation(
                                gt[:],
                                ps[:],
                                func,
                                bias=bias_sb[:, m : m + 1],
                            )
                            gts[g] = gt
                        flush_pend()
                        gi, gf, gg, go = (gts.get(g) for g in range(4))
                        if h8_prev is not None:
                            t1 = dp.tile([P, w], BF, tag="t1", bufs=1, name="t1_sb")
                            nc.vector.tensor_mul(t1[:], gi[:], gg[:])
                            t2 = dp.tile([P, w], F32, tag="t2", bufs=2, name="t2_sb")
                            nc.vector.tensor_mul(t2[:], gf[:], c_sb[:, r])
                            nc.vector.tensor_add(c_sb[:, r], t1[:], t2[:])
                        else:
                            nc.vector.tensor_mul(c_sb[:, r], gi[:], gg[:])
                        pend = (r, go)
                    flush_pend()

                    # heads: [mu(256); softplus_pre(256)] = Whd.T @ h.
                    # The sigma half runs first: its ACT chain (Exp -> Ln ->
                    # ze) is the long pole toward the next slot's x, and the
                    # Ln/ze ops overlap the mu head matmuls.
                    mu_sb = dp.tile([P, PO, w], F32, tag="mu", bufs=1, name="mu_sb")
                    sg_sb = dp.tile([P, PO, w], F32, tag="sg", bufs=1, name="sg_sb")
                    x_next = dp.tile([P, PO, w], BF, tag="x", bufs=2, name="x_sb")
                    x8_next = dp.tile([P, PO, w], F8, tag="x8", bufs=2, name="x8_sb")
                    last_slot = p_i == n_pass - 1 and t == n_slots - 1
                    ze = [None, None]
                    for hm in (PO, PO + 1, 0, 1):  # sigma halves, then mu
                        ps = pp.tile([P, w], F32, tag="ps", name="ps_h")
                        if hm >= PO:
                            # sigma head: fp8 DoubleRow over h8
                            po8 = hm - PO
                            for j in range(KH // 2):
                                for ci, (c0, cw) in enumerate(chunks):
                                    nc.tensor.matmul(
                                        ps[:, c0 : c0 + cw],
                                        wsp8_sb[
                                            :, 2 * j : 2 * j + 2,
                                            po8 * P : (po8 + 1) * P,
                                        ],
                                        h8_new[:, 2 * j : 2 * j + 2, c0 : c0 + cw],
                                        start=(j == 0),
                                        stop=(j == KH // 2 - 1),
                                        perf_mode=DR,
                                    )
                        else:
                            # mu head: graded output with small norm -> bf16
                            for k in range(KH):
                                for ci, (c0, cw) in enumerate(chunks):
                                    nc.tensor.matmul(
                                        ps[:, c0 : c0 + cw],
                                        whd_sb[:, k, hm * P : (hm + 1) * P],
                                        h_new[:, k, c0 : c0 + cw],
                                        start=(k == 0),
                                        stop=(k == KH - 1),
                                    )
                        if hm >= PO:
                            # softplus(u) = ln(1 + exp(u)); this toolchain has
                            # no softplus ACT table, but exp and ln share one.
                            po = hm - PO
                            nc.scalar.activation(
                                sg_sb[:, po],
                                ps[:],
                                AF.Exp,
                                bias=bias_sb[:, 36 + po : 37 + po],
                            )
                            nc.scalar.activation(
                                sg_sb[:, po], sg_sb[:, po], AF.Ln, bias=1.0
                            )
                            zt = dp.tile([P, w], F32, tag="t2", bufs=2, name="ze_sb")
                            nc.vector.tensor_mul(zt[:], sg_sb[:, po], eps_sb[:, po])
                            ze[po] = zt
                            nc.sync.dma_start(
                                osg_ext[t].rearrange("(po p) b -> p po b", p=P)[
                                    :, po : po + 1, col0 : col0 + w
                                ],
                                sg_sb[:, po : po + 1],
                            )
                        else:
                            po = hm
                            nc.scalar.activation(
                                mu_sb[:, po],
                                ps[:],
                                AF.Identity,
                                bias=bias_sb[:, 34 + po : 35 + po],
                            )
                            # z = mu + sigma*eps, overwriting the eps tile
                            nc.vector.tensor_add(
                                eps_sb[:, po], ze[po][:], mu_sb[:, po]
                            )
                            if not last_slot:
                                nc.vector.tensor_copy(x_next[:, po], eps_sb[:, po])
                                nc.vector.tensor_copy(x8_next[:, po], eps_sb[:, po])
                            nc.sync.dma_start(
                                oz_ext[t].rearrange("(po p) b -> p po b", p=P)[
                                    :, po : po + 1, col0 : col0 + w
                                ],
                                eps_sb[:, po : po + 1],
                            )
                            nc.sync.dma_start(
                                omu_ext[t].rearrange("(po p) b -> p po b", p=P)[
                                    :, po : po + 1, col0 : col0 + w
                                ],
                                mu_sb[:, po : po + 1],
                            )
                    x_cur = x_next
                    x8_cur = x8_next
                    h8_prev = h8_new
    return nc


_NC_CACHE = {}


def _get_nc(b_loc, w, n_slots):
    key = (b_loc, w, n_slots)
    if key not in _NC_CACHE:
        _NC_CACHE[key] = build(b_loc, w, n_slots)
    return _NC_CACHE[key]


def _prep_shared(We, be, W_ih, W_hh, b_ih, b_hh, W_mu, b_mu, W_sp, b_sp):
    f32 = np.float32
    wx_t = np.ascontiguousarray(np.asarray(W_ih, f32).T)
    wx = wx_t.astype(BF16)
    wx8 = wx_t.astype(FP8E4)
    wh = np.ascontiguousarray(np.asarray(W_hh, f32).T).astype(FP8E4)
    whd = np.ascontiguousarray(np.asarray(W_mu, f32).T).astype(BF16)
    wsp8 = np.ascontiguousarray(np.asarray(W_sp, f32).T).astype(FP8E4)
    we = np.ascontiguousarray(np.asarray(We, f32).T).astype(BF16)
    bg = (np.asarray(b_ih, f32) + np.asarray(b_hh, f32)).reshape(MT, P).T
    beT = np.asarray(be, f32).reshape(PO, P).T
    bmuT = np.asarray(b_mu, f32).reshape(PO, P).T
    bspT = np.asarray(b_sp, f32).reshape(PO, P).T
    bias = np.ascontiguousarray(
        np.concatenate([bg, beT, bmuT, bspT], axis=1), dtype=f32
    )
    return {
        "wx": wx, "wx8": wx8, "wh": wh, "whd": whd, "wsp8": wsp8,
        "we": we, "bias": bias,
    }


def _prep_in_maps(S, eps, shared, n_cores=N_CORES, b_loc=B_LOC):
    f32 = np.float32
    S = np.asarray(S, f32)
    eps = np.asarray(eps, f32)
    in_maps = []
    for ci in range(n_cores):
        rows = slice(ci * b_loc, (ci + 1) * b_loc)
        s_t = np.ascontiguousarray(S[rows].T).astype(BF16)
        eps_t = np.ascontiguousarray(eps[:NS, rows, :].transpose(0, 2, 1))
        in_maps.append({"s": s_t, "eps": eps_t, **shared})
    return in_maps


def _run(inputs, trace=False):
    from concourse.bass_utils import run_bass_kernel_spmd

    num_slots = int(inputs.get("num_slots", NS))
    nc = _get_nc(B_LOC, PASS_W, NS)
    shared = _prep_shared(
        inputs["We"], inputs["be"], inputs["W_ih"], inputs["W_hh"],
        inputs["b_ih"], inputs["b_hh"], inputs["W_mu"], inputs["b_mu"],
        inputs["W_sp"], inputs["b_sp"],
    )
    in_maps = _prep_in_maps(inputs["S"], inputs["eps"], shared)
    res = run_bass_kernel_spmd(
        nc, in_maps, core_ids=list(range(N_CORES)), trace=trace
    )
    zs = np.empty((NS, B, FEAT), np.float32)
    mus = np.empty((NS, B, FEAT), np.float32)
    sgs = np.empty((NS, B, FEAT), np.float32)
    for ci in range(N_CORES):
        rows = slice(ci * B_LOC, (ci + 1) * B_LOC)
        zs[:, rows, :] = res.results[ci]["oz"].transpose(0, 2, 1)
        mus[:, rows, :] = res.results[ci]["omu"].transpose(0, 2, 1)
        sgs[:, rows, :] = res.results[ci]["osg"].transpose(0, 2, 1)
    return (zs[:num_slots], mus[:num_slots], sgs[:num_slots]), res.exec_time_ns


def kernel(**inputs):
    out, _ = _run(inputs, trace=False)
    return out



# revision 34
# speedup vs baseline: 1.8386x; 1.0003x over previous
"""Trainium2 Bass kernel for nn_AutoregressivePrior (8-slot LSTM prior).

Strategy: pure data-parallel over batch (16384 rows -> 2048 per NeuronCore),
weights replicated. Feature-major dataflow on chip: every activation lives as
[feature_partition, batch_free] so LSTM matmul chains never transpose.

Most matmul FLOPs run in fp8-e4m3 with perf_mode=DoubleRow (2 fp8 weights
per PE cell -> 2x MAC rate; contraction pairs two 128-k-tiles per
instruction): the recurrent W_hh @ h (~70% of FLOPs; h's contribution to the
gates is small relative to x's, so its quantization error is diluted), the
W_ih @ x parts of the i/f/o gates (their error is compressed by saturating
sigmoids), and the softplus/sigma head. Precision-critical paths stay bf16:
the g-gate's W_ih @ x (feeds tanh content directly into the cell state), the
mu head (graded output with a small norm), and the encoder. Measured final
rel err ~1.2e-2 vs the 2e-2 budget. bf16 and DoubleRow-fp8 matmuls
accumulate into the same PSUM group (fp32). Gate nonlinearities fused with
bias on ScalarE straight out of PSUM, cell state held in fp16 in SBUF.

Per core the 2048-row batch is processed in two sequential passes of 1024
columns so all state fits in SBUF.

Inputs arrive as full-size numpy arrays; outputs are returned full-size
(zs, mus, sigmas) each [num_slots, 16384, 256] fp32, matching the reference.
"""

import sys

if "/opt/trn_rl_repo" not in sys.path:
    sys.path.insert(0, "/opt/trn_rl_repo")

import numpy as np
import ml_dtypes

BF16 = ml_dtypes.bfloat16
FP8E4 = ml_dtypes.float8_e4m3

B = 16384
N_CORES = 8
B_LOC = B // N_CORES  # 2048
SCENE = 256
FEAT = 256
HID = 1024
G4 = 4 * HID  # 4096
NS = 8
P = 128
PO = FEAT // P  # 2
KH = HID // P  # 8
KX = FEAT // P  # 2
MT = G4 // P  # 32
HM = (2 * FEAT) // P  # 4 head m-tiles (mu 2 + softplus 2)

PASS_W = 1024  # batch columns per pass on chip

_PATCHED = False


def _patch_tile_drain():
    """walrus in this toolchain rejects >1 sync-wait on a single instruction;
    split excess waits onto standalone single-wait EventSemaphore instructions
    that run on the same engine immediately before the original instruction."""
    global _PATCHED
    if _PATCHED:
        return
    import bass_rust
    import concourse.tile as tile
    from concourse import mybir
    from concourse.vector_clock import ScopedClock

    MAXW = 1
    _orig_lower = tile.TileContext._lower_ordered_insts

    def _lower_split_waits(self, ordered):
        nc = self.nc
        for bbn, insts in ordered.items():
            out = []
            for inst in insts:
                si = getattr(inst, "sync_info", None)
                if si is not None:
                    waits = list(si.on_wait)
                    if len(waits) > MAXW:
                        imm = [w for w in waits if w.wait_mode == "sem-ge-imm"]
                        other = [w for w in waits if w.wait_mode != "sem-ge-imm"]
                        assert len(other) <= MAXW, (inst.name, waits)
                        keep_n = MAXW - len(other)
                        if keep_n > 0:
                            move = imm[: len(imm) - keep_n]
                            keep = imm[len(imm) - keep_n :]
                        else:
                            move = imm
                            keep = []
                        for wt in move:
                            wi = mybir.InstEventSemaphore(
                                name=nc.get_next_instruction_name(),
                                ins=[],
                                outs=[],
                                engine=inst.engine,
                            )
                            wi.sync_info = bass_rust.SyncInfo(
                                on_wait=[wt], on_update=[]
                            )
                            out.append(wi)
                        si.on_wait = other + keep
                out.append(inst)
            insts[:] = out
        return _orig_lower(self, ordered)

    tile.TileContext._lower_ordered_insts = _lower_split_waits

    def _drain_and_barrier(self, tick_clock, wait_clock):
        nc = self.nc
        drain_inst = nc.sync.drain()
        wait_clock.add_sem_waits(
            drain_inst.ins, ScopedClock({None: tick_clock.global_clock})
        )
        si = drain_inst.ins.sync_info
        if si is not None and len(si.on_wait) > 1:
            waits = list(si.on_wait)
            si.on_wait = waits[:1]
            name2handle = {h.name: h for h in self.sems.allocated().values()}
            for w in waits[1:]:
                assert w.wait_mode == "sem-ge-imm", w
                nc.sync.wait_ge(name2handle[w.ant_name], w.wait_value)
        nc.all_engine_barrier()
        popped = nc._tile_sem_poison_stack.pop()
        assert popped is self._sem_poison
        nc.clear_and_free_semaphores(list(self.sems.allocated().values()))
        nc.all_engine_barrier()

    tile.TileContext._drain_and_barrier = _drain_and_barrier
    _PATCHED = True


def build(b_loc=B_LOC, w=PASS_W, n_slots=NS, mm_n=512):
    _patch_tile_drain()
    import concourse.bass as bass
    import concourse.tile as tile
    from concourse import mybir

    F32 = mybir.dt.float32
    BF = mybir.dt.bfloat16
    F16 = mybir.dt.float16
    F8 = mybir.dt.float8e4
    AF = mybir.ActivationFunctionType
    DR = mybir.MatmulPerfMode.DoubleRow

    n_pass = b_loc // w
    assert n_pass * w == b_loc
    chunks = [(c, min(mm_n, w - c)) for c in range(0, w, mm_n)]

    nc = bass.Bass()
    s_ext = nc.dram_tensor("s", [SCENE, b_loc], BF, kind="ExternalInput")
    eps_ext = nc.dram_tensor("eps", [NS, FEAT, b_loc], F32, kind="ExternalInput")
    wx_ext = nc.dram_tensor("wx", [FEAT, G4], BF, kind="ExternalInput")
    wx8_ext = nc.dram_tensor("wx8", [FEAT, G4], F8, kind="ExternalInput")
    wh_ext = nc.dram_tensor("wh", [HID, G4], F8, kind="ExternalInput")
    whd_ext = nc.dram_tensor("whd", [HID, FEAT], BF, kind="ExternalInput")
    wsp8_ext = nc.dram_tensor("wsp8", [HID, FEAT], F8, kind="ExternalInput")
    we_ext = nc.dram_tensor("we", [SCENE, FEAT], BF, kind="ExternalInput")
    bias_ext = nc.dram_tensor("bias", [P, 38], F32, kind="ExternalInput")
    oz_ext = nc.dram_tensor("oz", [NS, FEAT, b_loc], F32, kind="ExternalOutput")
    omu_ext = nc.dram_tensor("omu", [NS, FEAT, b_loc], F32, kind="ExternalOutput")
    osg_ext = nc.dram_tensor("osg", [NS, FEAT, b_loc], F32, kind="ExternalOutput")

    with tile.TileContext(nc) as tc:
        with (
            tc.tile_pool(name="wp", bufs=1) as wp,
            tc.tile_pool(name="work", bufs=2) as dp,
            tc.tile_pool(name="psum", bufs=4, space="PSUM") as pp,
        ):
            # DMA order matters: the encoder and slot 0 need only we/wx/bias,
            # so load those before the big (8MB) W_hh to let PE start early.
            we_sb = wp.tile([P, KX, FEAT], BF, tag="we", name="we_sb")
            nc.sync.dma_start(we_sb[:], we_ext.rearrange("(ko p) m -> p ko m", p=P))
            bias_sb = wp.tile([P, 38], F32, tag="bias", name="bias_sb")
            nc.sync.dma_start(bias_sb[:], bias_ext[:])
            # wx/wh/whd are not needed until slot-0 gates / heads(0) /
            # gates(1); their DMAs are emitted after pass 0's s DMA so the
            # encoder's inputs don't queue behind 11MB of weight traffic.
            wx_sb = {
                gq: wp.tile([P, KX, HID], BF, tag=f"wx{gq}", name=f"wx{gq}_sb")
                for gq in (0, 2, 3)
            }
            wx8_sb = wp.tile([P, KX, G4], F8, tag="wx8", name="wx8_sb")
            whd_sb = wp.tile([P, KH, FEAT], BF, tag="whd", name="whd_sb")
            wsp8_sb = wp.tile([P, KH, FEAT], F8, tag="wsp8", name="wsp8_sb")
            wh_sb = wp.tile([P, KH, G4], F8, tag="wh", name="wh_sb")

            # Both passes' S halves are prefetched up front (per matmul-chunk
            # tiles, so the encoder's first chunk starts as soon as its half
            # lands) and pass 1's encoder never queues behind pass 0's
            # output DMAs.
            s_sbs = []
            for p_i in range(n_pass):
                halves = []
                for ci, (c0, cw) in enumerate(chunks):
                    s_sb = dp.tile(
                        [P, KX, cw], BF, tag=f"s{ci}", bufs=n_pass, name="s_sb"
                    )
                    nc.sync.dma_start(
                        s_sb[:],
                        s_ext.rearrange("(po p) b -> p po b", p=P)[
                            :, :, p_i * w + c0 : p_i * w + c0 + cw
                        ],
                    )
                    halves.append(s_sb)
                s_sbs.append(halves)
            # the f-gate's bf16 x-part is never used (slot 0 skips f, later
            # slots run it in fp8), so quarter m8..15 of wx is never loaded;
            # per-gate tiles let slot 0's g0 matmuls start after 512KB
            for gq in (0, 2, 3):
                nc.sync.dma_start(
                    wx_sb[gq][:],
                    wx_ext.rearrange("(ko p) m -> p ko m", p=P)[
                        :, :, gq * HID : (gq + 1) * HID
                    ],
                )
            nc.sync.dma_start(wx8_sb[:], wx8_ext.rearrange("(ko p) m -> p ko m", p=P))
            nc.sync.dma_start(whd_sb[:], whd_ext.rearrange("(ko p) m -> p ko m", p=P))
            nc.sync.dma_start(
                wsp8_sb[:], wsp8_ext.rearrange("(ko p) m -> p ko m", p=P)
            )
            nc.sync.dma_start(wh_sb[:], wh_ext.rearrange("(ko p) m -> p ko m", p=P))

            for p_i in range(n_pass):
                col0 = p_i * w
                c_sb = dp.tile([P, KH, w], F16, tag="c", bufs=1, name="c_sb")
                s_sb = s_sbs[p_i]

                # encoder: x0 = gelu(We @ S_loc.T + be), feature-major
                x_cur = dp.tile([P, PO, w], BF, tag="x", bufs=2, name="x0_sb")
                for fi in range(PO):
                    ps = pp.tile([P, w], F32, tag="ps", name="ps_enc")
                    for ci, (c0, cw) in enumerate(chunks):
                        for k in range(KX):
                            nc.tensor.matmul(
                                ps[:, c0 : c0 + cw],
                                we_sb[:, k, fi * P : (fi + 1) * P],
                                s_sb[ci][:, k, :],
                                start=(k == 0),
                                stop=(k == KX - 1),
                            )
                    nc.scalar.activation(
                        x_cur[:, fi],
                        ps[:],
                        AF.Gelu,
                        bias=bias_sb[:, 32 + fi : 33 + fi],
                    )

                h8_prev = None
                x8_cur = None
                for t in range(n_slots):
                    eps_sb = dp.tile([P, PO, w], F32, tag="eps", bufs=2, name="eps_sb")
                    nc.sync.dma_start(
                        eps_sb[:],
                        eps_ext[t].rearrange("(po p) b -> p po b", p=P)[
                            :, :, col0 : col0 + w
                        ],
                    )
                    h_new = dp.tile([P, KH, w], BF, tag="h", bufs=2, name="h_sb")
                    # fp8 copy of h feeds this slot's DoubleRow sigma-head
                    # matmuls and the next slot's DoubleRow gate matmuls; the
                    # bf16 h feeds this slot's mu-head matmuls.
                    h8_new = dp.tile([P, KH, w], F8, tag="h8", bufs=2, name="h8_sb")
                    # (r_idx, o_gate_tile) whose tanh(c)/h-multiply is
                    # deferred until after the NEXT r-block's gate evictions,
                    # so ScalarE's eviction stream never stalls on the DVE
                    # c-update (that stall starves PSUM recycling and PE).
                    pend = None

                    def flush_pend():
                        nonlocal pend
                        if pend is None:
                            return
                        rp, gop = pend
                        th = dp.tile([P, w], BF, tag="th", bufs=2, name="th_sb")
                        nc.scalar.activation(th[:], c_sb[:, rp], AF.Tanh)
                        nc.vector.tensor_mul(h_new[:, rp], gop[:], th[:])
                        nc.vector.tensor_copy(h8_new[:, rp], h_new[:, rp])
                        pend = None

                    for r in range(KH):
                        gts = {}
                        # slot 0: c=0, so the forget gate is never used
                        gate_ids = (0, 2, 3) if h8_prev is None else (0, 1, 2, 3)
                        for g in gate_ids:
                            m = g * KH + r
                            ps = pp.tile([P, w], F32, tag="ps", name="ps_g")
                            if h8_prev is not None:
                                for j in range(KH // 2):
                                    for ci, (c0, cw) in enumerate(chunks):
                                        nc.tensor.matmul(
                                            ps[:, c0 : c0 + cw],
                                            wh_sb[
                                                :, 2 * j : 2 * j + 2,
                                                m * P : (m + 1) * P,
                                            ],
                                            h8_prev[:, 2 * j : 2 * j + 2, c0 : c0 + cw],
                                            start=(j == 0),
                                            stop=False,
                                            perf_mode=DR,
                                        )
                                if g != 2:
                                    # i/f/o x-part: one DoubleRow fp8 matmul
                                    # pairing both 128-k-tiles of x
                                    for ci, (c0, cw) in enumerate(chunks):
                                        nc.tensor.matmul(
                                            ps[:, c0 : c0 + cw],
                                            wx8_sb[:, 0:KX, m * P : (m + 1) * P],
                                            x8_cur[:, 0:KX, c0 : c0 + cw],
                                            start=False,
                                            stop=True,
                                            perf_mode=DR,
                                        )
                                else:
                                    # g-gate x-part feeds tanh content
                                    # directly -> keep bf16
                                    for k in range(KX):
                                        for ci, (c0, cw) in enumerate(chunks):
                                            nc.tensor.matmul(
                                                ps[:, c0 : c0 + cw],
                                                wx_sb[g][:, k, r * P : (r + 1) * P],
                                                x_cur[:, k, c0 : c0 + cw],
                                                start=False,
                                                stop=(k == KX - 1),
                                            )
                            else:
                                for k in range(KX):
                                    for ci, (c0, cw) in enumerate(chunks):
                                        nc.tensor.matmul(
                                            ps[:, c0 : c0 + cw],
                                            wx_sb[g][:, k, r * P : (r + 1) * P],
                                            x_cur[:, k, c0 : c0 + cw],
                                            start=(k == 0),
                                            stop=(k == KX - 1),
                                        )
                            # o-gate is double-buffered: its consumer (the
                            # deferred h-multiply) runs one r-block late
                            gt = dp.tile(
                                [P, w],
                                BF,
                                tag=f"g{g}",
                                bufs=2 if g == 3 else 1,
                                name=f"g{g}_sb",
                            )
                            func = AF.Tanh if g == 2 else AF.Sigmoid
                            nc.scalar.activation(
                                gt[:],
                                ps[:],
                                func,
                                bias=bias_sb[:, m : m + 1],
                            )
                            gts[g] = gt
                        flush_pend()
                        gi, gf, gg, go = (gts.get(g) for g in range(4))
                        if h8_prev is not None:
                            t1 = dp.tile([P, w], BF, tag="t1", bufs=1, name="t1_sb")
                            nc.vector.tensor_mul(t1[:], gi[:], gg[:])
                            t2 = dp.tile([P, w], F32, tag="t2", bufs=2, name="t2_sb")
                            nc.vector.tensor_mul(t2[:], gf[:], c_sb[:, r])
                            nc.vector.tensor_add(c_sb[:, r], t1[:], t2[:])
                        else:
                            nc.vector.tensor_mul(c_sb[:, r], gi[:], gg[:])
                        pend = (r, go)
                    flush_pend()

                    # heads: mu = Wmu.T @ h (bf16), sigma = softplus head
                    # (fp8 DoubleRow over h8).
                    mu_sb = dp.tile([P, PO, w], F32, tag="mu", bufs=1, name="mu_sb")
                    sg_sb = dp.tile([P, PO, w], F32, tag="sg", bufs=1, name="sg_sb")
                    last = t == n_slots - 1

                    def emit_sigma_mm(po8, ps):
                        for j in range(KH // 2):
                            for ci, (c0, cw) in enumerate(chunks):
                                nc.tensor.matmul(
                                    ps[:, c0 : c0 + cw],
                                    wsp8_sb[
                                        :, 2 * j : 2 * j + 2, po8 * P : (po8 + 1) * P
                                    ],
                                    h8_new[:, 2 * j : 2 * j + 2, c0 : c0 + cw],
                                    start=(j == 0),
                                    stop=(j == KH // 2 - 1),
                                    perf_mode=DR,
                                )

                    def emit_sigma_act(po, ps):
                        # softplus(u) = ln(1 + exp(u)); this toolchain has
                        # no softplus ACT table, but exp and ln share one.
                        nc.scalar.activation(
                            sg_sb[:, po],
                            ps[:],
                            AF.Exp,
                            bias=bias_sb[:, 36 + po : 37 + po],
                        )
                        nc.scalar.activation(
                            sg_sb[:, po], sg_sb[:, po], AF.Ln, bias=1.0
                        )
                        zt = dp.tile([P, w], F32, tag="t2", bufs=2, name="ze_sb")
                        nc.vector.tensor_mul(zt[:], sg_sb[:, po], eps_sb[:, po])
                        nc.sync.dma_start(
                            osg_ext[t].rearrange("(po p) b -> p po b", p=P)[
                                :, po : po + 1, col0 : col0 + w
                            ],
                            sg_sb[:, po : po + 1],
                        )
                        return zt

                    def emit_mu_act(po, ps, zt, x_next, x8_next):
                        nc.scalar.activation(
                            mu_sb[:, po],
                            ps[:],
                            AF.Identity,
                            bias=bias_sb[:, 34 + po : 35 + po],
                        )
                        # z = mu + sigma*eps, overwriting the eps tile
                        nc.vector.tensor_add(eps_sb[:, po], zt[:], mu_sb[:, po])
                        if x_next is not None:
                            nc.vector.tensor_copy(x_next[:, po], eps_sb[:, po])
                            nc.vector.tensor_copy(x8_next[:, po], eps_sb[:, po])
                        nc.sync.dma_start(
                            oz_ext[t].rearrange("(po p) b -> p po b", p=P)[
                                :, po : po + 1, col0 : col0 + w
                            ],
                            eps_sb[:, po : po + 1],
                        )
                        nc.sync.dma_start(
                            omu_ext[t].rearrange("(po p) b -> p po b", p=P)[
                                :, po : po + 1, col0 : col0 + w
                            ],
                            mu_sb[:, po : po + 1],
                        )

                    if not last:
                        # sigma first: its ACT chain (Exp -> Ln -> ze) is the
                        # long pole toward the next slot's x, and the Ln/ze
                        # ops overlap the mu head matmuls.
                        x_next = dp.tile([P, PO, w], BF, tag="x", bufs=2, name="x_sb")
                        x8_next = dp.tile(
                            [P, PO, w], F8, tag="x8", bufs=2, name="x8_sb"
                        )
                        ze = [None, None]
                        for hm in (PO, PO + 1, 0, 1):
                            ps = pp.tile([P, w], F32, tag="ps", name="ps_h")
                            if hm >= PO:
                                emit_sigma_mm(hm - PO, ps)
                                ze[hm - PO] = emit_sigma_act(hm - PO, ps)
                            else:
                                for k in range(KH):
                                    for ci, (c0, cw) in enumerate(chunks):
                                        nc.tensor.matmul(
                                            ps[:, c0 : c0 + cw],
                                            whd_sb[:, k, hm * P : (hm + 1) * P],
                                            h_new[:, k, c0 : c0 + cw],
                                            start=(k == 0),
                                            stop=(k == KH - 1),
                                        )
                                emit_mu_act(hm, ps, ze[hm], x_next, x8_next)
                        x_cur = x_next
                        x8_cur = x8_next
                    else:
                        # last slot: no next-slot gates to buffer the
                        # r=7 eviction/tanh/h serial chain, so run the mu
                        # heads first and k-outer -- their k<=6 matmuls only
                        # need earlier r-blocks' h and overlap that chain;
                        # sigma (which needs the full h8) follows.
                        ps_mu = [
                            pp.tile([P, w], F32, tag="ps", name="ps_h")
                            for _ in range(PO)
                        ]
                        for k in range(KH):
                            for po in range(PO):
                                for ci, (c0, cw) in enumerate(chunks):
                                    nc.tensor.matmul(
                                        ps_mu[po][:, c0 : c0 + cw],
                                        whd_sb[:, k, po * P : (po + 1) * P],
                                        h_new[:, k, c0 : c0 + cw],
                                        start=(k == 0),
                                        stop=(k == KH - 1),
                                    )
                        for po in range(PO):
                            ps = pp.tile([P, w], F32, tag="ps", name="ps_h")
                            emit_sigma_mm(po, ps)
                            zt = emit_sigma_act(po, ps)
                            emit_mu_act(po, ps_mu[po], zt, None, None)
                    h8_prev = h8_new
    return nc


_NC_CACHE = {}


def _get_nc(b_loc, w, n_slots):
    key = (b_loc, w, n_slots)
    if key not in _NC_CACHE:
        _NC_CACHE[key] = build(b_loc, w, n_slots)
    return _NC_CACHE[key]


def _prep_shared(We, be, W_ih, W_hh, b_ih, b_hh, W_mu, b_mu, W_sp, b_sp):
    f32 = np.float32
    wx_t = np.ascontiguousarray(np.asarray(W_ih, f32).T)
    wx = wx_t.astype(BF16)
    wx8 = wx_t.astype(FP8E4)
    wh = np.ascontiguousarray(np.asarray(W_hh, f32).T).astype(FP8E4)
    whd = np.ascontiguousarray(np.asarray(W_mu, f32).T).astype(BF16)
    wsp8 = np.ascontiguousarray(np.asarray(W_sp, f32).T).astype(FP8E4)
    we = np.ascontiguousarray(np.asarray(We, f32).T).astype(BF16)
    bg = (np.asarray(b_ih, f32) + np.asarray(b_hh, f32)).reshape(MT, P).T
    beT = np.asarray(be, f32).reshape(PO, P).T
    bmuT = np.asarray(b_mu, f32).reshape(PO, P).T
    bspT = np.asarray(b_sp, f32).reshape(PO, P).T
    bias = np.ascontiguousarray(
        np.concatenate([bg, beT, bmuT, bspT], axis=1), dtype=f32
    )
    return {
        "wx": wx, "wx8": wx8, "wh": wh, "whd": whd, "wsp8": wsp8,
        "we": we, "bias": bias,
    }


def _prep_in_maps(S, eps, shared, n_cores=N_CORES, b_loc=B_LOC):
    f32 = np.float32
    S = np.asarray(S, f32)
    eps = np.asarray(eps, f32)
    in_maps = []
    for ci in range(n_cores):
        rows = slice(ci * b_loc, (ci + 1) * b_loc)
        s_t = np.ascontiguousarray(S[rows].T).astype(BF16)
        eps_t = np.ascontiguousarray(eps[:NS, rows, :].transpose(0, 2, 1))
        in_maps.append({"s": s_t, "eps": eps_t, **shared})
    return in_maps


def _run(inputs, trace=False):
    from concourse.bass_utils import run_bass_kernel_spmd

    num_slots = int(inputs.get("num_slots", NS))
    nc = _get_nc(B_LOC, PASS_W, NS)
    shared = _prep_shared(
        inputs["We"], inputs["be"], inputs["W_ih"], inputs["W_hh"],
        inputs["b_ih"], inputs["b_hh"], inputs["W_mu"], inputs["b_mu"],
        inputs["W_sp"], inputs["b_sp"],
    )
    in_maps = _prep_in_maps(inputs["S"], inputs["eps"], shared)
    res = run_bass_kernel_spmd(
        nc, in_maps, core_ids=list(range(N_CORES)), trace=trace
    )
    zs = np.empty((NS, B, FEAT), np.float32)
    mus = np.empty((NS, B, FEAT), np.float32)
    sgs = np.empty((NS, B, FEAT), np.float32)
    for ci in range(N_CORES):
        rows = slice(ci * B_LOC, (ci + 1) * B_LOC)
        zs[:, rows, :] = res.results[ci]["oz"].transpose(0, 2, 1)
        mus[:, rows, :] = res.results[ci]["omu"].transpose(0, 2, 1)
        sgs[:, rows, :] = res.results[ci]["osg"].transpose(0, 2, 1)
    return (zs[:num_slots], mus[:num_slots], sgs[:num_slots]), res.exec_time_ns


def kernel(**inputs):
    out, _ = _run(inputs, trace=False)
    return out



# revision 38
# speedup vs baseline: 1.8387x; 1.0000x over previous
"""Trainium2 Bass kernel for nn_AutoregressivePrior (8-slot LSTM prior).

Strategy: pure data-parallel over batch (16384 rows -> 2048 per NeuronCore),
weights replicated. Feature-major dataflow on chip: every activation lives as
[feature_partition, batch_free] so LSTM matmul chains never transpose.

Most matmul FLOPs run in fp8-e4m3 with perf_mode=DoubleRow (2 fp8 weights
per PE cell -> 2x MAC rate; contraction pairs two 128-k-tiles per
instruction): the recurrent W_hh @ h (~70% of FLOPs; h's contribution to the
gates is small relative to x's, so its quantization error is diluted), the
W_ih @ x parts of the i/f/o gates (their error is compressed by saturating
sigmoids), and the softplus/sigma head. Precision-critical paths stay bf16:
the g-gate's W_ih @ x (feeds tanh content directly into the cell state), the
mu head (graded output with a small norm), and the encoder. Measured final
rel err ~1.2e-2 vs the 2e-2 budget. bf16 and DoubleRow-fp8 matmuls
accumulate into the same PSUM group (fp32). Gate nonlinearities fused with
bias on ScalarE straight out of PSUM, cell state held in fp16 in SBUF.

Per core the 2048-row batch is processed in two sequential passes of 1024
columns so all state fits in SBUF.

Inputs arrive as full-size numpy arrays; outputs are returned full-size
(zs, mus, sigmas) each [num_slots, 16384, 256] fp32, matching the reference.
"""

import sys

if "/opt/trn_rl_repo" not in sys.path:
    sys.path.insert(0, "/opt/trn_rl_repo")

import numpy as np
import ml_dtypes

BF16 = ml_dtypes.bfloat16
FP8E4 = ml_dtypes.float8_e4m3

B = 16384
N_CORES = 8
B_LOC = B // N_CORES  # 2048
SCENE = 256
FEAT = 256
HID = 1024
G4 = 4 * HID  # 4096
NS = 8
P = 128
PO = FEAT // P  # 2
KH = HID // P  # 8
KX = FEAT // P  # 2
MT = G4 // P  # 32
HM = (2 * FEAT) // P  # 4 head m-tiles (mu 2 + softplus 2)

PASS_W = 1024  # batch columns per pass on chip

_PATCHED = False


def _patch_tile_drain():
    """walrus in this toolchain rejects >1 sync-wait on a single instruction;
    split excess waits onto standalone single-wait EventSemaphore instructions
    that run on the same engine immediately before the original instruction."""
    global _PATCHED
    if _PATCHED:
        return
    import bass_rust
    import concourse.tile as tile
    from concourse import mybir
    from concourse.vector_clock import ScopedClock

    MAXW = 1
    _orig_lower = tile.TileContext._lower_ordered_insts

    def _lower_split_waits(self, ordered):
        nc = self.nc
        for bbn, insts in ordered.items():
            out = []
            for inst in insts:
                si = getattr(inst, "sync_info", None)
                if si is not None:
                    waits = list(si.on_wait)
                    if len(waits) > MAXW:
                        imm = [w for w in waits if w.wait_mode == "sem-ge-imm"]
                        other = [w for w in waits if w.wait_mode != "sem-ge-imm"]
                        assert len(other) <= MAXW, (inst.name, waits)
                        keep_n = MAXW - len(other)
                        if keep_n > 0:
                            move = imm[: len(imm) - keep_n]
                            keep = imm[len(imm) - keep_n :]
                        else:
                            move = imm
                            keep = []
                        for wt in move:
                            wi = mybir.InstEventSemaphore(
                                name=nc.get_next_instruction_name(),
                                ins=[],
                                outs=[],
                                engine=inst.engine,
                            )
                            wi.sync_info = bass_rust.SyncInfo(
                                on_wait=[wt], on_update=[]
                            )
                            out.append(wi)
                        si.on_wait = other + keep
                out.append(inst)
            insts[:] = out
        return _orig_lower(self, ordered)

    tile.TileContext._lower_ordered_insts = _lower_split_waits

    def _drain_and_barrier(self, tick_clock, wait_clock):
        nc = self.nc
        drain_inst = nc.sync.drain()
        wait_clock.add_sem_waits(
            drain_inst.ins, ScopedClock({None: tick_clock.global_clock})
        )
        si = drain_inst.ins.sync_info
        if si is not None and len(si.on_wait) > 1:
            waits = list(si.on_wait)
            si.on_wait = waits[:1]
            name2handle = {h.name: h for h in self.sems.allocated().values()}
            for w in waits[1:]:
                assert w.wait_mode == "sem-ge-imm", w
                nc.sync.wait_ge(name2handle[w.ant_name], w.wait_value)
        nc.all_engine_barrier()
        popped = nc._tile_sem_poison_stack.pop()
        assert popped is self._sem_poison
        nc.clear_and_free_semaphores(list(self.sems.allocated().values()))
        nc.all_engine_barrier()

    tile.TileContext._drain_and_barrier = _drain_and_barrier
    _PATCHED = True


def build(b_loc=B_LOC, w=PASS_W, n_slots=NS, mm_n=512):
    _patch_tile_drain()
    import concourse.bass as bass
    import concourse.tile as tile
    from concourse import mybir

    F32 = mybir.dt.float32
    BF = mybir.dt.bfloat16
    F16 = mybir.dt.float16
    F8 = mybir.dt.float8e4
    AF = mybir.ActivationFunctionType
    DR = mybir.MatmulPerfMode.DoubleRow

    n_pass = b_loc // w
    assert n_pass * w == b_loc
    chunks = [(c, min(mm_n, w - c)) for c in range(0, w, mm_n)]

    nc = bass.Bass()
    s_ext = nc.dram_tensor("s", [SCENE, b_loc], BF, kind="ExternalInput")
    eps_ext = nc.dram_tensor("eps", [NS, FEAT, b_loc], F32, kind="ExternalInput")
    wx_ext = nc.dram_tensor("wx", [FEAT, G4], BF, kind="ExternalInput")
    wx8_ext = nc.dram_tensor("wx8", [FEAT, G4], F8, kind="ExternalInput")
    wh_ext = nc.dram_tensor("wh", [HID, G4], F8, kind="ExternalInput")
    whd_ext = nc.dram_tensor("whd", [HID, FEAT], BF, kind="ExternalInput")
    wsp8_ext = nc.dram_tensor("wsp8", [HID, FEAT], F8, kind="ExternalInput")
    we_ext = nc.dram_tensor("we", [SCENE, FEAT], BF, kind="ExternalInput")
    bias_ext = nc.dram_tensor("bias", [P, 38], F32, kind="ExternalInput")
    oz_ext = nc.dram_tensor("oz", [NS, FEAT, b_loc], F32, kind="ExternalOutput")
    omu_ext = nc.dram_tensor("omu", [NS, FEAT, b_loc], F32, kind="ExternalOutput")
    osg_ext = nc.dram_tensor("osg", [NS, FEAT, b_loc], F32, kind="ExternalOutput")

    with tile.TileContext(nc) as tc:
        with (
            tc.tile_pool(name="wp", bufs=1) as wp,
            tc.tile_pool(name="work", bufs=2) as dp,
            tc.tile_pool(name="psum", bufs=4, space="PSUM") as pp,
        ):
            # DMA order matters: the encoder needs only s/we/bias, so those
            # go first (s half 0 leads — it gates the first matmul); slot-0
            # weights next; the big W_hh last.
            we_sb = wp.tile([P, KX, FEAT], BF, tag="we", name="we_sb")
            bias_sb = wp.tile([P, 38], F32, tag="bias", name="bias_sb")
            wx_sb = {
                gq: wp.tile([P, KX, HID], BF, tag=f"wx{gq}", name=f"wx{gq}_sb")
                for gq in (0, 2, 3)
            }
            wx8_sb = wp.tile([P, KX, G4], F8, tag="wx8", name="wx8_sb")
            whd_sb = wp.tile([P, KH, FEAT], BF, tag="whd", name="whd_sb")
            wsp8_sb = wp.tile([P, KH, FEAT], F8, tag="wsp8", name="wsp8_sb")
            wh_sb = wp.tile([P, KH, G4], F8, tag="wh", name="wh_sb")

            # Both passes' S halves are prefetched up front (per matmul-chunk
            # tiles, so the encoder's first chunk starts as soon as its half
            # lands) and pass 1's encoder never queues behind pass 0's
            # output DMAs.
            s_sbs = [
                [
                    dp.tile([P, KX, cw], BF, tag=f"s{ci}", bufs=n_pass, name="s_sb")
                    for ci, (c0, cw) in enumerate(chunks)
                ]
                for p_i in range(n_pass)
            ]

            def dma_s(p_i):
                for ci, (c0, cw) in enumerate(chunks):
                    nc.sync.dma_start(
                        s_sbs[p_i][ci][:],
                        s_ext.rearrange("(po p) b -> p po b", p=P)[
                            :, :, p_i * w + c0 : p_i * w + c0 + cw
                        ],
                    )

            dma_s(0)
            nc.sync.dma_start(we_sb[:], we_ext.rearrange("(ko p) m -> p ko m", p=P))
            nc.sync.dma_start(bias_sb[:], bias_ext[:])
            for p_i in range(1, n_pass):
                dma_s(p_i)
            # the f-gate's bf16 x-part is never used (slot 0 skips f, later
            # slots run it in fp8), so quarter m8..15 of wx is never loaded;
            # per-gate tiles let slot 0's g0 matmuls start after 512KB
            for gq in (0, 2, 3):
                nc.sync.dma_start(
                    wx_sb[gq][:],
                    wx_ext.rearrange("(ko p) m -> p ko m", p=P)[
                        :, :, gq * HID : (gq + 1) * HID
                    ],
                )
            nc.sync.dma_start(wx8_sb[:], wx8_ext.rearrange("(ko p) m -> p ko m", p=P))
            nc.sync.dma_start(whd_sb[:], whd_ext.rearrange("(ko p) m -> p ko m", p=P))
            nc.sync.dma_start(
                wsp8_sb[:], wsp8_ext.rearrange("(ko p) m -> p ko m", p=P)
            )
            nc.sync.dma_start(wh_sb[:], wh_ext.rearrange("(ko p) m -> p ko m", p=P))

            for p_i in range(n_pass):
                col0 = p_i * w
                c_sb = dp.tile([P, KH, w], F16, tag="c", bufs=1, name="c_sb")
                s_sb = s_sbs[p_i]

                # encoder: x0 = gelu(We @ S_loc.T + be), feature-major
                x_cur = dp.tile([P, PO, w], BF, tag="x", bufs=2, name="x0_sb")
                for fi in range(PO):
                    ps = pp.tile([P, w], F32, tag="ps", name="ps_enc")
                    for ci, (c0, cw) in enumerate(chunks):
                        for k in range(KX):
                            nc.tensor.matmul(
                                ps[:, c0 : c0 + cw],
                                we_sb[:, k, fi * P : (fi + 1) * P],
                                s_sb[ci][:, k, :],
                                start=(k == 0),
                                stop=(k == KX - 1),
                            )
                    nc.scalar.activation(
                        x_cur[:, fi],
                        ps[:],
                        AF.Gelu,
                        bias=bias_sb[:, 32 + fi : 33 + fi],
                    )

                h8_prev = None
                x8_cur = None
                for t in range(n_slots):
                    eps_sb = dp.tile([P, PO, w], F32, tag="eps", bufs=2, name="eps_sb")
                    nc.sync.dma_start(
                        eps_sb[:],
                        eps_ext[t].rearrange("(po p) b -> p po b", p=P)[
                            :, :, col0 : col0 + w
                        ],
                    )
                    h_new = dp.tile([P, KH, w], BF, tag="h", bufs=2, name="h_sb")
                    # fp8 copy of h feeds this slot's DoubleRow sigma-head
                    # matmuls and the next slot's DoubleRow gate matmuls; the
                    # bf16 h feeds this slot's mu-head matmuls.
                    h8_new = dp.tile([P, KH, w], F8, tag="h8", bufs=2, name="h8_sb")
                    # (r_idx, o_gate_tile) whose tanh(c)/h-multiply is
                    # deferred until after the NEXT r-block's gate evictions,
                    # so ScalarE's eviction stream never stalls on the DVE
                    # c-update (that stall starves PSUM recycling and PE).
                    pend = None

                    def flush_pend():
                        nonlocal pend
                        if pend is None:
                            return
                        rp, gop = pend
                        th = dp.tile([P, w], BF, tag="th", bufs=2, name="th_sb")
                        nc.scalar.activation(th[:], c_sb[:, rp], AF.Tanh)
                        nc.vector.tensor_mul(h_new[:, rp], gop[:], th[:])
                        nc.vector.tensor_copy(h8_new[:, rp], h_new[:, rp])
                        pend = None

                    for r in range(KH):
                        gts = {}
                        # slot 0: c=0, so the forget gate is never used
                        gate_ids = (0, 2, 3) if h8_prev is None else (0, 1, 2, 3)
                        for g in gate_ids:
                            m = g * KH + r
                            ps = pp.tile([P, w], F32, tag="ps", name="ps_g")
                            if h8_prev is not None:
                                for j in range(KH // 2):
                                    for ci, (c0, cw) in enumerate(chunks):
                                        nc.tensor.matmul(
                                            ps[:, c0 : c0 + cw],
                                            wh_sb[
                                                :, 2 * j : 2 * j + 2,
                                                m * P : (m + 1) * P,
                                            ],
                                            h8_prev[:, 2 * j : 2 * j + 2, c0 : c0 + cw],
                                            start=(j == 0),
                                            stop=False,
                                            perf_mode=DR,
                                        )
                                if g != 2:
                                    # i/f/o x-part: one DoubleRow fp8 matmul
                                    # pairing both 128-k-tiles of x
                                    for ci, (c0, cw) in enumerate(chunks):
                                        nc.tensor.matmul(
                                            ps[:, c0 : c0 + cw],
                                            wx8_sb[:, 0:KX, m * P : (m + 1) * P],
                                            x8_cur[:, 0:KX, c0 : c0 + cw],
                                            start=False,
                                            stop=True,
                                            perf_mode=DR,
                                        )
                                else:
                                    # g-gate x-part feeds tanh content
                                    # directly -> keep bf16
                                    for k in range(KX):
                                        for ci, (c0, cw) in enumerate(chunks):
                                            nc.tensor.matmul(
                                                ps[:, c0 : c0 + cw],
                                                wx_sb[g][:, k, r * P : (r + 1) * P],
                                                x_cur[:, k, c0 : c0 + cw],
                                                start=False,
                                                stop=(k == KX - 1),
                                            )
                            else:
                                for k in range(KX):
                                    for ci, (c0, cw) in enumerate(chunks):
                                        nc.tensor.matmul(
                                            ps[:, c0 : c0 + cw],
                                            wx_sb[g][:, k, r * P : (r + 1) * P],
                                            x_cur[:, k, c0 : c0 + cw],
                                            start=(k == 0),
                                            stop=(k == KX - 1),
                                        )
                            # o-gate is double-buffered: its consumer (the
                            # deferred h-multiply) runs one r-block late
                            gt = dp.tile(
                                [P, w],
                                BF,
                                tag=f"g{g}",
                                bufs=2 if g == 3 else 1,
                                name=f"g{g}_sb",
                            )
                            func = AF.Tanh if g == 2 else AF.Sigmoid
                            nc.scalar.activation(
                                gt[:],
                                ps[:],
                                func,
                                bias=bias_sb[:, m : m + 1],
                            )
                            gts[g] = gt
                        flush_pend()
                        gi, gf, gg, go = (gts.get(g) for g in range(4))
                        if h8_prev is not None:
                            t1 = dp.tile([P, w], BF, tag="t1", bufs=1, name="t1_sb")
                            nc.vector.tensor_mul(t1[:], gi[:], gg[:])
                            t2 = dp.tile([P, w], F32, tag="t2", bufs=2, name="t2_sb")
                            nc.vector.tensor_mul(t2[:], gf[:], c_sb[:, r])
                            nc.vector.tensor_add(c_sb[:, r], t1[:], t2[:])
                        else:
                            nc.vector.tensor_mul(c_sb[:, r], gi[:], gg[:])
                        pend = (r, go)
                    flush_pend()

                    # heads: mu = Wmu.T @ h (bf16), sigma = softplus head
                    # (fp8 DoubleRow over h8).
                    mu_sb = dp.tile([P, PO, w], F32, tag="mu", bufs=1, name="mu_sb")
                    sg_sb = dp.tile([P, PO, w], F32, tag="sg", bufs=1, name="sg_sb")
                    last = t == n_slots - 1

                    def emit_sigma_mm(po8, ps):
                        for j in range(KH // 2):
                            for ci, (c0, cw) in enumerate(chunks):
                                nc.tensor.matmul(
                                    ps[:, c0 : c0 + cw],
                                    wsp8_sb[
                                        :, 2 * j : 2 * j + 2, po8 * P : (po8 + 1) * P
                                    ],
                                    h8_new[:, 2 * j : 2 * j + 2, c0 : c0 + cw],
                                    start=(j == 0),
                                    stop=(j == KH // 2 - 1),
                                    perf_mode=DR,
                                )

                    def emit_sigma_act(po, ps):
                        # softplus(u) = ln(1 + exp(u)); this toolchain has
                        # no softplus ACT table, but exp and ln share one.
                        nc.scalar.activation(
                            sg_sb[:, po],
                            ps[:],
                            AF.Exp,
                            bias=bias_sb[:, 36 + po : 37 + po],
                        )
                        nc.scalar.activation(
                            sg_sb[:, po], sg_sb[:, po], AF.Ln, bias=1.0
                        )
                        zt = dp.tile([P, w], F32, tag="t2", bufs=2, name="ze_sb")
                        nc.vector.tensor_mul(zt[:], sg_sb[:, po], eps_sb[:, po])
                        nc.sync.dma_start(
                            osg_ext[t].rearrange("(po p) b -> p po b", p=P)[
                                :, po : po + 1, col0 : col0 + w
                            ],
                            sg_sb[:, po : po + 1],
                        )
                        return zt

                    def emit_mu_evict(po, ps):
                        nc.scalar.activation(
                            mu_sb[:, po],
                            ps[:],
                            AF.Identity,
                            bias=bias_sb[:, 34 + po : 35 + po],
                        )
                        nc.sync.dma_start(
                            omu_ext[t].rearrange("(po p) b -> p po b", p=P)[
                                :, po : po + 1, col0 : col0 + w
                            ],
                            mu_sb[:, po : po + 1],
                        )

                    def emit_z(po, zt, x_next, x8_next):
                        # z = mu + sigma*eps, overwriting the eps tile
                        nc.vector.tensor_add(eps_sb[:, po], zt[:], mu_sb[:, po])
                        if x_next is not None:
                            nc.vector.tensor_copy(x_next[:, po], eps_sb[:, po])
                            nc.vector.tensor_copy(x8_next[:, po], eps_sb[:, po])
                        nc.sync.dma_start(
                            oz_ext[t].rearrange("(po p) b -> p po b", p=P)[
                                :, po : po + 1, col0 : col0 + w
                            ],
                            eps_sb[:, po : po + 1],
                        )

                    if not last:
                        # sigma first: its ACT chain (Exp -> Ln -> ze) is the
                        # long pole toward the next slot's x, and the Ln/ze
                        # ops overlap the mu head matmuls.
                        x_next = dp.tile([P, PO, w], BF, tag="x", bufs=2, name="x_sb")
                        x8_next = dp.tile(
                            [P, PO, w], F8, tag="x8", bufs=2, name="x8_sb"
                        )
                        ze = [None, None]
                        for hm in (PO, PO + 1, 0, 1):
                            ps = pp.tile([P, w], F32, tag="ps", name="ps_h")
                            if hm >= PO:
                                emit_sigma_mm(hm - PO, ps)
                                ze[hm - PO] = emit_sigma_act(hm - PO, ps)
                            else:
                                for k in range(KH):
                                    for ci, (c0, cw) in enumerate(chunks):
                                        nc.tensor.matmul(
                                            ps[:, c0 : c0 + cw],
                                            whd_sb[:, k, hm * P : (hm + 1) * P],
                                            h_new[:, k, c0 : c0 + cw],
                                            start=(k == 0),
                                            stop=(k == KH - 1),
                                        )
                                emit_mu_evict(hm, ps)
                                emit_z(hm, ze[hm], x_next, x8_next)
                        x_cur = x_next
                        x8_cur = x8_next
                    else:
                        # last slot: no next-slot gates to buffer the
                        # r=7 eviction/tanh/h serial chain, so run the mu
                        # heads first and k-outer -- their k<=6 matmuls only
                        # need earlier r-blocks' h and overlap that chain;
                        # sigma (which needs the full h8) follows. The mu
                        # PSUMs are evicted BEFORE the sigma ACT chain so
                        # the ring frees for the next pass's encoder.
                        ps_mu = [
                            pp.tile([P, w], F32, tag="ps", name="ps_h")
                            for _ in range(PO)
                        ]
                        for k in range(KH):
                            for po in range(PO):
                                for ci, (c0, cw) in enumerate(chunks):
                                    nc.tensor.matmul(
                                        ps_mu[po][:, c0 : c0 + cw],
                                        whd_sb[:, k, po * P : (po + 1) * P],
                                        h_new[:, k, c0 : c0 + cw],
                                        start=(k == 0),
                                        stop=(k == KH - 1),
                                    )
                        for po in range(PO):
                            emit_mu_evict(po, ps_mu[po])
                        for po in range(PO):
                            ps = pp.tile([P, w], F32, tag="ps", name="ps_h")
                            emit_sigma_mm(po, ps)
                            zt = emit_sigma_act(po, ps)
                            emit_z(po, zt, None, None)
                    h8_prev = h8_new
    return nc


_NC_CACHE = {}


def _get_nc(b_loc, w, n_slots):
    key = (b_loc, w, n_slots)
    if key not in _NC_CACHE:
        _NC_CACHE[key] = build(b_loc, w, n_slots)
    return _NC_CACHE[key]


def _prep_shared(We, be, W_ih, W_hh, b_ih, b_hh, W_mu, b_mu, W_sp, b_sp):
    f32 = np.float32
    wx_t = np.ascontiguousarray(np.asarray(W_ih, f32).T)
    wx = wx_t.astype(BF16)
    wx8 = wx_t.astype(FP8E4)
    wh = np.ascontiguousarray(np.asarray(W_hh, f32).T).astype(FP8E4)
    whd = np.ascontiguousarray(np.asarray(W_mu, f32).T).astype(BF16)
    wsp8 = np.ascontiguousarray(np.asarray(W_sp, f32).T).astype(FP8E4)
    we = np.ascontiguousarray(np.asarray(We, f32).T).astype(BF16)
    bg = (np.asarray(b_ih, f32) + np.asarray(b_hh, f32)).reshape(MT, P).T
    beT = np.asarray(be, f32).reshape(PO, P).T
    bmuT = np.asarray(b_mu, f32).reshape(PO, P).T
    bspT = np.asarray(b_sp, f32).reshape(PO, P).T
    bias = np.ascontiguousarray(
        np.concatenate([bg, beT, bmuT, bspT], axis=1), dtype=f32
    )
    return {
        "wx": wx, "wx8": wx8, "wh": wh, "whd": whd, "wsp8": wsp8,
        "we": we, "bias": bias,
    }


def _prep_in_maps(S, eps, shared, n_cores=N_CORES, b_loc=B_LOC):
    f32 = np.float32
    S = np.asarray(S, f32)
    eps = np.asarray(eps, f32)
    in_maps = []
    for ci in range(n_cores):
        rows = slice(ci * b_loc, (ci + 1) * b_loc)
        s_t = np.ascontiguousarray(S[rows].T).astype(BF16)
        eps_t = np.ascontiguousarray(eps[:NS, rows, :].transpose(0, 2, 1))
        in_maps.append({"s": s_t, "eps": eps_t, **shared})
    return in_maps


def _run(inputs, trace=False):
    from concourse.bass_utils import run_bass_kernel_spmd

    num_slots = int(inputs.get("num_slots", NS))
    nc = _get_nc(B_LOC, PASS_W, NS)
    shared = _prep_shared(
        inputs["We"], inputs["be"], inputs["W_ih"], inputs["W_hh"],
        inputs["b_ih"], inputs["b_hh"], inputs["W_mu"], inputs["b_mu"],
        inputs["W_sp"], inputs["b_sp"],
    )
    in_maps = _prep_in_maps(inputs["S"], inputs["eps"], shared)
    res = run_bass_kernel_spmd(
        nc, in_maps, core_ids=list(range(N_CORES)), trace=trace
    )
    zs = np.empty((NS, B, FEAT), np.float32)
    mus = np.empty((NS, B, FEAT), np.float32)
    sgs = np.empty((NS, B, FEAT), np.float32)
    for ci in range(N_CORES):
        rows = slice(ci * B_LOC, (ci + 1) * B_LOC)
        zs[:, rows, :] = res.results[ci]["oz"].transpose(0, 2, 1)
        mus[:, rows, :] = res.results[ci]["omu"].transpose(0, 2, 1)
        sgs[:, rows, :] = res.results[ci]["osg"].transpose(0, 2, 1)
    return (zs[:num_slots], mus[:num_slots], sgs[:num_slots]), res.exec_time_ns


def kernel(**inputs):
    out, _ = _run(inputs, trace=False)
    return out



# revision 40
# speedup vs baseline: 1.8394x; 1.0004x over previous
"""Trainium2 Bass kernel for nn_AutoregressivePrior (8-slot LSTM prior).

Strategy: pure data-parallel over batch (16384 rows -> 2048 per NeuronCore),
weights replicated. Feature-major dataflow on chip: every activation lives as
[feature_partition, batch_free] so LSTM matmul chains never transpose.

Most matmul FLOPs run in fp8-e4m3 with perf_mode=DoubleRow (2 fp8 weights
per PE cell -> 2x MAC rate; contraction pairs two 128-k-tiles per
instruction): the recurrent W_hh @ h (~70% of FLOPs; h's contribution to the
gates is small relative to x's, so its quantization error is diluted), the
W_ih @ x parts of the i/f/o gates (their error is compressed by saturating
sigmoids), and the softplus/sigma head. Precision-critical paths stay bf16:
the g-gate's W_ih @ x (feeds tanh content directly into the cell state), the
mu head (graded output with a small norm), and the encoder. Measured final
rel err ~1.2e-2 vs the 2e-2 budget. bf16 and DoubleRow-fp8 matmuls
accumulate into the same PSUM group (fp32). Gate nonlinearities fused with
bias on ScalarE straight out of PSUM, cell state held in fp16 in SBUF.

Per core the 2048-row batch is processed in two sequential passes of 1024
columns so all state fits in SBUF.

Inputs arrive as full-size numpy arrays; outputs are returned full-size
(zs, mus, sigmas) each [num_slots, 16384, 256] fp32, matching the reference.
"""

import sys

if "/opt/trn_rl_repo" not in sys.path:
    sys.path.insert(0, "/opt/trn_rl_repo")

import numpy as np
import ml_dtypes

BF16 = ml_dtypes.bfloat16
FP8E4 = ml_dtypes.float8_e4m3

B = 16384
N_CORES = 8
B_LOC = B // N_CORES  # 2048
SCENE = 256
FEAT = 256
HID = 1024
G4 = 4 * HID  # 4096
NS = 8
P = 128
PO = FEAT // P  # 2
KH = HID // P  # 8
KX = FEAT // P  # 2
MT = G4 // P  # 32
HM = (2 * FEAT) // P  # 4 head m-tiles (mu 2 + softplus 2)

PASS_W = 1024  # batch columns per pass on chip

_PATCHED = False


def _patch_tile_drain():
    """walrus in this toolchain rejects >1 sync-wait on a single instruction;
    split excess waits onto standalone single-wait EventSemaphore instructions
    that run on the same engine immediately before the original instruction."""
    global _PATCHED
    if _PATCHED:
        return
    import bass_rust
    import concourse.tile as tile
    from concourse import mybir
    from concourse.vector_clock import ScopedClock

    MAXW = 1
    _orig_lower = tile.TileContext._lower_ordered_insts

    def _lower_split_waits(self, ordered):
        nc = self.nc
        for bbn, insts in ordered.items():
            out = []
            for inst in insts:
                si = getattr(inst, "sync_info", None)
                if si is not None:
                    waits = list(si.on_wait)
                    if len(waits) > MAXW:
                        imm = [w for w in waits if w.wait_mode == "sem-ge-imm"]
                        other = [w for w in waits if w.wait_mode != "sem-ge-imm"]
                        assert len(other) <= MAXW, (inst.name, waits)
                        keep_n = MAXW - len(other)
                        if keep_n > 0:
                            move = imm[: len(imm) - keep_n]
                            keep = imm[len(imm) - keep_n :]
                        else:
                            move = imm
                            keep = []
                        for wt in move:
                            wi = mybir.InstEventSemaphore(
                                name=nc.get_next_instruction_name(),
                                ins=[],
                                outs=[],
                                engine=inst.engine,
                            )
                            wi.sync_info = bass_rust.SyncInfo(
                                on_wait=[wt], on_update=[]
                            )
                            out.append(wi)
                        si.on_wait = other + keep
                out.append(inst)
            insts[:] = out
        return _orig_lower(self, ordered)

    tile.TileContext._lower_ordered_insts = _lower_split_waits

    def _drain_and_barrier(self, tick_clock, wait_clock):
        nc = self.nc
        drain_inst = nc.sync.drain()
        wait_clock.add_sem_waits(
            drain_inst.ins, ScopedClock({None: tick_clock.global_clock})
        )
        si = drain_inst.ins.sync_info
        if si is not None and len(si.on_wait) > 1:
            waits = list(si.on_wait)
            si.on_wait = waits[:1]
            name2handle = {h.name: h for h in self.sems.allocated().values()}
            for w in waits[1:]:
                assert w.wait_mode == "sem-ge-imm", w
                nc.sync.wait_ge(name2handle[w.ant_name], w.wait_value)
        nc.all_engine_barrier()
        popped = nc._tile_sem_poison_stack.pop()
        assert popped is self._sem_poison
        nc.clear_and_free_semaphores(list(self.sems.allocated().values()))
        nc.all_engine_barrier()

    tile.TileContext._drain_and_barrier = _drain_and_barrier
    _PATCHED = True


def build(b_loc=B_LOC, w=PASS_W, n_slots=NS, mm_n=512):
    _patch_tile_drain()
    import concourse.bass as bass
    import concourse.tile as tile
    from concourse import mybir

    F32 = mybir.dt.float32
    BF = mybir.dt.bfloat16
    F16 = mybir.dt.float16
    F8 = mybir.dt.float8e4
    AF = mybir.ActivationFunctionType
    DR = mybir.MatmulPerfMode.DoubleRow

    n_pass = b_loc // w
    assert n_pass * w == b_loc
    chunks = [(c, min(mm_n, w - c)) for c in range(0, w, mm_n)]

    nc = bass.Bass()
    s_ext = nc.dram_tensor("s", [SCENE, b_loc], BF, kind="ExternalInput")
    eps_ext = nc.dram_tensor("eps", [NS, FEAT, b_loc], F32, kind="ExternalInput")
    wx_ext = nc.dram_tensor("wx", [FEAT, G4], BF, kind="ExternalInput")
    wx8_ext = nc.dram_tensor("wx8", [FEAT, G4], F8, kind="ExternalInput")
    wh_ext = nc.dram_tensor("wh", [HID, G4], F8, kind="ExternalInput")
    whd_ext = nc.dram_tensor("whd", [HID, FEAT], BF, kind="ExternalInput")
    wsp8_ext = nc.dram_tensor("wsp8", [HID, FEAT], F8, kind="ExternalInput")
    we_ext = nc.dram_tensor("we", [SCENE, FEAT], BF, kind="ExternalInput")
    bias_ext = nc.dram_tensor("bias", [P, 38], F32, kind="ExternalInput")
    oz_ext = nc.dram_tensor("oz", [NS, FEAT, b_loc], F32, kind="ExternalOutput")
    omu_ext = nc.dram_tensor("omu", [NS, FEAT, b_loc], F32, kind="ExternalOutput")
    osg_ext = nc.dram_tensor("osg", [NS, FEAT, b_loc], F32, kind="ExternalOutput")

    with tile.TileContext(nc) as tc:
        with (
            tc.tile_pool(name="wp", bufs=1) as wp,
            tc.tile_pool(name="work", bufs=2) as dp,
            tc.tile_pool(name="psum", bufs=4, space="PSUM") as pp,
        ):
            # DMA order matters: the encoder needs only s/we/bias, so those
            # go first (s half 0 leads — it gates the first matmul); slot-0
            # weights next; the big W_hh last.
            we_sb = wp.tile([P, KX, FEAT], BF, tag="we", name="we_sb")
            bias_sb = wp.tile([P, 38], F32, tag="bias", name="bias_sb")
            wx_sb = {
                gq: wp.tile([P, KX, HID], BF, tag=f"wx{gq}", name=f"wx{gq}_sb")
                for gq in (0, 2, 3)
            }
            wx8_sb = wp.tile([P, KX, G4], F8, tag="wx8", name="wx8_sb")
            whd_sb = wp.tile([P, KH, FEAT], BF, tag="whd", name="whd_sb")
            wsp8_sb = wp.tile([P, KH, FEAT], F8, tag="wsp8", name="wsp8_sb")
            wh_sb = wp.tile([P, KH, G4], F8, tag="wh", name="wh_sb")

            # Both passes' S halves are prefetched up front (per matmul-chunk
            # tiles, so the encoder's first chunk starts as soon as its half
            # lands) and pass 1's encoder never queues behind pass 0's
            # output DMAs.
            s_sbs = [
                [
                    dp.tile([P, KX, cw], BF, tag=f"s{ci}", bufs=n_pass, name="s_sb")
                    for ci, (c0, cw) in enumerate(chunks)
                ]
                for p_i in range(n_pass)
            ]

            def dma_s(p_i):
                for ci, (c0, cw) in enumerate(chunks):
                    nc.sync.dma_start(
                        s_sbs[p_i][ci][:],
                        s_ext.rearrange("(po p) b -> p po b", p=P)[
                            :, :, p_i * w + c0 : p_i * w + c0 + cw
                        ],
                    )

            dma_s(0)
            nc.sync.dma_start(we_sb[:], we_ext.rearrange("(ko p) m -> p ko m", p=P))
            nc.sync.dma_start(bias_sb[:], bias_ext[:])
            for p_i in range(1, n_pass):
                dma_s(p_i)
            # the f-gate's bf16 x-part is never used (slot 0 skips f, later
            # slots run it in fp8), so quarter m8..15 of wx is never loaded;
            # per-gate tiles let slot 0's g0 matmuls start after 512KB
            for gq in (0, 2, 3):
                nc.sync.dma_start(
                    wx_sb[gq][:],
                    wx_ext.rearrange("(ko p) m -> p ko m", p=P)[
                        :, :, gq * HID : (gq + 1) * HID
                    ],
                )
            nc.sync.dma_start(wx8_sb[:], wx8_ext.rearrange("(ko p) m -> p ko m", p=P))
            nc.sync.dma_start(whd_sb[:], whd_ext.rearrange("(ko p) m -> p ko m", p=P))
            nc.sync.dma_start(
                wsp8_sb[:], wsp8_ext.rearrange("(ko p) m -> p ko m", p=P)
            )
            nc.sync.dma_start(wh_sb[:], wh_ext.rearrange("(ko p) m -> p ko m", p=P))

            def emit_encoder(p_i):
                # encoder: x0 = gelu(We @ S_loc.T + be), feature-major
                x0 = dp.tile([P, PO, w], BF, tag="x", bufs=2, name="x0_sb")
                for fi in range(PO):
                    ps = pp.tile([P, w], F32, tag="ps", name="ps_enc")
                    for ci, (c0, cw) in enumerate(chunks):
                        for k in range(KX):
                            nc.tensor.matmul(
                                ps[:, c0 : c0 + cw],
                                we_sb[:, k, fi * P : (fi + 1) * P],
                                s_sbs[p_i][ci][:, k, :],
                                start=(k == 0),
                                stop=(k == KX - 1),
                            )
                    nc.scalar.activation(
                        x0[:, fi],
                        ps[:],
                        AF.Gelu,
                        bias=bias_sb[:, 32 + fi : 33 + fi],
                    )
                return x0

            x_enc = emit_encoder(0)
            for p_i in range(n_pass):
                col0 = p_i * w
                c_sb = dp.tile([P, KH, w], F16, tag="c", bufs=1, name="c_sb")
                x_cur = x_enc

                h8_prev = None
                x8_cur = None
                for t in range(n_slots):
                    eps_sb = dp.tile([P, PO, w], F32, tag="eps", bufs=2, name="eps_sb")
                    nc.sync.dma_start(
                        eps_sb[:],
                        eps_ext[t].rearrange("(po p) b -> p po b", p=P)[
                            :, :, col0 : col0 + w
                        ],
                    )
                    h_new = dp.tile([P, KH, w], BF, tag="h", bufs=2, name="h_sb")
                    # fp8 copy of h feeds this slot's DoubleRow sigma-head
                    # matmuls and the next slot's DoubleRow gate matmuls; the
                    # bf16 h feeds this slot's mu-head matmuls.
                    h8_new = dp.tile([P, KH, w], F8, tag="h8", bufs=2, name="h8_sb")
                    # (r_idx, o_gate_tile) whose tanh(c)/h-multiply is
                    # deferred until after the NEXT r-block's gate evictions,
                    # so ScalarE's eviction stream never stalls on the DVE
                    # c-update (that stall starves PSUM recycling and PE).
                    pend = None

                    def flush_pend():
                        nonlocal pend
                        if pend is None:
                            return
                        rp, gop = pend
                        th = dp.tile([P, w], BF, tag="th", bufs=2, name="th_sb")
                        nc.scalar.activation(th[:], c_sb[:, rp], AF.Tanh)
                        nc.vector.tensor_mul(h_new[:, rp], gop[:], th[:])
                        nc.vector.tensor_copy(h8_new[:, rp], h_new[:, rp])
                        pend = None

                    for r in range(KH):
                        gts = {}
                        # slot 0: c=0, so the forget gate is never used
                        gate_ids = (0, 2, 3) if h8_prev is None else (0, 1, 2, 3)
                        for g in gate_ids:
                            m = g * KH + r
                            ps = pp.tile([P, w], F32, tag="ps", name="ps_g")
                            if h8_prev is not None:
                                for j in range(KH // 2):
                                    for ci, (c0, cw) in enumerate(chunks):
                                        nc.tensor.matmul(
                                            ps[:, c0 : c0 + cw],
                                            wh_sb[
                                                :, 2 * j : 2 * j + 2,
                                                m * P : (m + 1) * P,
                                            ],
                                            h8_prev[:, 2 * j : 2 * j + 2, c0 : c0 + cw],
                                            start=(j == 0),
                                            stop=False,
                                            perf_mode=DR,
                                        )
                                if g != 2:
                                    # i/f/o x-part: one DoubleRow fp8 matmul
                                    # pairing both 128-k-tiles of x
                                    for ci, (c0, cw) in enumerate(chunks):
                                        nc.tensor.matmul(
                                            ps[:, c0 : c0 + cw],
                                            wx8_sb[:, 0:KX, m * P : (m + 1) * P],
                                            x8_cur[:, 0:KX, c0 : c0 + cw],
                                            start=False,
                                            stop=True,
                                            perf_mode=DR,
                                        )
                                else:
                                    # g-gate x-part feeds tanh content
                                    # directly -> keep bf16
                                    for k in range(KX):
                                        for ci, (c0, cw) in enumerate(chunks):
                                            nc.tensor.matmul(
                                                ps[:, c0 : c0 + cw],
                                                wx_sb[g][:, k, r * P : (r + 1) * P],
                                                x_cur[:, k, c0 : c0 + cw],
                                                start=False,
                                                stop=(k == KX - 1),
                                            )
                            else:
                                for k in range(KX):
                                    for ci, (c0, cw) in enumerate(chunks):
                                        nc.tensor.matmul(
                                            ps[:, c0 : c0 + cw],
                                            wx_sb[g][:, k, r * P : (r + 1) * P],
                                            x_cur[:, k, c0 : c0 + cw],
                                            start=(k == 0),
                                            stop=(k == KX - 1),
                                        )
                            # o-gate is double-buffered: its consumer (the
                            # deferred h-multiply) runs one r-block late
                            gt = dp.tile(
                                [P, w],
                                BF,
                                tag=f"g{g}",
                                bufs=2 if g == 3 else 1,
                                name=f"g{g}_sb",
                            )
                            func = AF.Tanh if g == 2 else AF.Sigmoid
                            nc.scalar.activation(
                                gt[:],
                                ps[:],
                                func,
                                bias=bias_sb[:, m : m + 1],
                            )
                            gts[g] = gt
                        flush_pend()
                        gi, gf, gg, go = (gts.get(g) for g in range(4))
                        if h8_prev is not None:
                            t1 = dp.tile([P, w], BF, tag="t1", bufs=1, name="t1_sb")
                            nc.vector.tensor_mul(t1[:], gi[:], gg[:])
                            t2 = dp.tile([P, w], F32, tag="t2", bufs=2, name="t2_sb")
                            nc.vector.tensor_mul(t2[:], gf[:], c_sb[:, r])
                            nc.vector.tensor_add(c_sb[:, r], t1[:], t2[:])
                        else:
                            nc.vector.tensor_mul(c_sb[:, r], gi[:], gg[:])
                        pend = (r, go)
                    flush_pend()

                    # Next pass's encoder is emitted between the last slot's
                    # gates and heads: its matmuls reuse early-released gate
                    # PSUM ring slots and fill the PE window while the r=7
                    # eviction/tanh/h chain drains (the heads need full h).
                    if t == n_slots - 1 and p_i + 1 < n_pass:
                        x_enc = emit_encoder(p_i + 1)

                    # heads: mu = Wmu.T @ h (bf16), sigma = softplus head
                    # (fp8 DoubleRow over h8).
                    mu_sb = dp.tile([P, PO, w], F32, tag="mu", bufs=1, name="mu_sb")
                    sg_sb = dp.tile([P, PO, w], F32, tag="sg", bufs=1, name="sg_sb")
                    last = t == n_slots - 1

                    def emit_sigma_mm(po8, ps):
                        for j in range(KH // 2):
                            for ci, (c0, cw) in enumerate(chunks):
                                nc.tensor.matmul(
                                    ps[:, c0 : c0 + cw],
                                    wsp8_sb[
                                        :, 2 * j : 2 * j + 2, po8 * P : (po8 + 1) * P
                                    ],
                                    h8_new[:, 2 * j : 2 * j + 2, c0 : c0 + cw],
                                    start=(j == 0),
                                    stop=(j == KH // 2 - 1),
                                    perf_mode=DR,
                                )

                    def emit_sigma_act(po, ps):
                        # softplus(u) = ln(1 + exp(u)); this toolchain has
                        # no softplus ACT table, but exp and ln share one.
                        nc.scalar.activation(
                            sg_sb[:, po],
                            ps[:],
                            AF.Exp,
                            bias=bias_sb[:, 36 + po : 37 + po],
                        )
                        nc.scalar.activation(
                            sg_sb[:, po], sg_sb[:, po], AF.Ln, bias=1.0
                        )
                        zt = dp.tile([P, w], F32, tag="t2", bufs=2, name="ze_sb")
                        nc.vector.tensor_mul(zt[:], sg_sb[:, po], eps_sb[:, po])
                        nc.sync.dma_start(
                            osg_ext[t].rearrange("(po p) b -> p po b", p=P)[
                                :, po : po + 1, col0 : col0 + w
                            ],
                            sg_sb[:, po : po + 1],
                        )
                        return zt

                    def emit_mu_evict(po, ps):
                        nc.scalar.activation(
                            mu_sb[:, po],
                            ps[:],
                            AF.Identity,
                            bias=bias_sb[:, 34 + po : 35 + po],
                        )
                        nc.sync.dma_start(
                            omu_ext[t].rearrange("(po p) b -> p po b", p=P)[
                                :, po : po + 1, col0 : col0 + w
                            ],
                            mu_sb[:, po : po + 1],
                        )

                    def emit_z(po, zt, x_next, x8_next):
                        # z = mu + sigma*eps, overwriting the eps tile
                        nc.vector.tensor_add(eps_sb[:, po], zt[:], mu_sb[:, po])
                        if x_next is not None:
                            nc.vector.tensor_copy(x_next[:, po], eps_sb[:, po])
                            nc.vector.tensor_copy(x8_next[:, po], eps_sb[:, po])
                        nc.sync.dma_start(
                            oz_ext[t].rearrange("(po p) b -> p po b", p=P)[
                                :, po : po + 1, col0 : col0 + w
                            ],
                            eps_sb[:, po : po + 1],
                        )

                    if not last:
                        # sigma first: its ACT chain (Exp -> Ln -> ze) is the
                        # long pole toward the next slot's x, and the Ln/ze
                        # ops overlap the mu head matmuls.
                        x_next = dp.tile([P, PO, w], BF, tag="x", bufs=2, name="x_sb")
                        x8_next = dp.tile(
                            [P, PO, w], F8, tag="x8", bufs=2, name="x8_sb"
                        )
                        ze = [None, None]
                        for hm in (PO, PO + 1, 0, 1):
                            ps = pp.tile([P, w], F32, tag="ps", name="ps_h")
                            if hm >= PO:
                                emit_sigma_mm(hm - PO, ps)
                                ze[hm - PO] = emit_sigma_act(hm - PO, ps)
                            else:
                                for k in range(KH):
                                    for ci, (c0, cw) in enumerate(chunks):
                                        nc.tensor.matmul(
                                            ps[:, c0 : c0 + cw],
                                            whd_sb[:, k, hm * P : (hm + 1) * P],
                                            h_new[:, k, c0 : c0 + cw],
                                            start=(k == 0),
                                            stop=(k == KH - 1),
                                        )
                                emit_mu_evict(hm, ps)
                                emit_z(hm, ze[hm], x_next, x8_next)
                        x_cur = x_next
                        x8_cur = x8_next
                    else:
                        # last slot: no next-slot gates to buffer the
                        # r=7 eviction/tanh/h serial chain, so run the mu
                        # heads first and k-outer -- their k<=6 matmuls only
                        # need earlier r-blocks' h and overlap that chain;
                        # sigma (which needs the full h8) follows. The mu
                        # PSUMs are evicted BEFORE the sigma ACT chain so
                        # the ring frees for the next pass's encoder.
                        ps_mu = [
                            pp.tile([P, w], F32, tag="ps", name="ps_h")
                            for _ in range(PO)
                        ]
                        for k in range(KH):
                            for po in range(PO):
                                for ci, (c0, cw) in enumerate(chunks):
                                    nc.tensor.matmul(
                                        ps_mu[po][:, c0 : c0 + cw],
                                        whd_sb[:, k, po * P : (po + 1) * P],
                                        h_new[:, k, c0 : c0 + cw],
                                        start=(k == 0),
                                        stop=(k == KH - 1),
                                    )
                        for po in range(PO):
                            emit_mu_evict(po, ps_mu[po])
                        for po in range(PO):
                            ps = pp.tile([P, w], F32, tag="ps", name="ps_h")
                            emit_sigma_mm(po, ps)
                            zt = emit_sigma_act(po, ps)
                            emit_z(po, zt, None, None)
                    h8_prev = h8_new
    return nc


_NC_CACHE = {}


def _get_nc(b_loc, w, n_slots):
    key = (b_loc, w, n_slots)
    if key not in _NC_CACHE:
        _NC_CACHE[key] = build(b_loc, w, n_slots)
    return _NC_CACHE[key]


def _prep_shared(We, be, W_ih, W_hh, b_ih, b_hh, W_mu, b_mu, W_sp, b_sp):
    f32 = np.float32
    wx_t = np.ascontiguousarray(np.asarray(W_ih, f32).T)
    wx = wx_t.astype(BF16)
    wx8 = wx_t.astype(FP8E4)
    wh = np.ascontiguousarray(np.asarray(W_hh, f32).T).astype(FP8E4)
    whd = np.ascontiguousarray(np.asarray(W_mu, f32).T).astype(BF16)
    wsp8 = np.ascontiguousarray(np.asarray(W_sp, f32).T).astype(FP8E4)
    we = np.ascontiguousarray(np.asarray(We, f32).T).astype(BF16)
    bg = (np.asarray(b_ih, f32) + np.asarray(b_hh, f32)).reshape(MT, P).T
    beT = np.asarray(be, f32).reshape(PO, P).T
    bmuT = np.asarray(b_mu, f32).reshape(PO, P).T
    bspT = np.asarray(b_sp, f32).reshape(PO, P).T
    bias = np.ascontiguousarray(
        np.concatenate([bg, beT, bmuT, bspT], axis=1), dtype=f32
    )
    return {
        "wx": wx, "wx8": wx8, "wh": wh, "whd": whd, "wsp8": wsp8,
        "we": we, "bias": bias,
    }


def _prep_in_maps(S, eps, shared, n_cores=N_CORES, b_loc=B_LOC):
    f32 = np.float32
    S = np.asarray(S, f32)
    eps = np.asarray(eps, f32)
    in_maps = []
    for ci in range(n_cores):
        rows = slice(ci * b_loc, (ci + 1) * b_loc)
        s_t = np.ascontiguousarray(S[rows].T).astype(BF16)
        eps_t = np.ascontiguousarray(eps[:NS, rows, :].transpose(0, 2, 1))
        in_maps.append({"s": s_t, "eps": eps_t, **shared})
    return in_maps


def _run(inputs, trace=False):
    from concourse.bass_utils import run_bass_kernel_spmd

    num_slots = int(inputs.get("num_slots", NS))
    nc = _get_nc(B_LOC, PASS_W, NS)
    shared = _prep_shared(
        inputs["We"], inputs["be"], inputs["W_ih"], inputs["W_hh"],
        inputs["b_ih"], inputs["b_hh"], inputs["W_mu"], inputs["b_mu"],
        inputs["W_sp"], inputs["b_sp"],
    )
    in_maps = _prep_in_maps(inputs["S"], inputs["eps"], shared)
    res = run_bass_kernel_spmd(
        nc, in_maps, core_ids=list(range(N_CORES)), trace=trace
    )
    zs = np.empty((NS, B, FEAT), np.float32)
    mus = np.empty((NS, B, FEAT), np.float32)
    sgs = np.empty((NS, B, FEAT), np.float32)
    for ci in range(N_CORES):
        rows = slice(ci * B_LOC, (ci + 1) * B_LOC)
        zs[:, rows, :] = res.results[ci]["oz"].transpose(0, 2, 1)
        mus[:, rows, :] = res.results[ci]["omu"].transpose(0, 2, 1)
        sgs[:, rows, :] = res.results[ci]["osg"].transpose(0, 2, 1)
    return (zs[:num_slots], mus[:num_slots], sgs[:num_slots]), res.exec_time_ns


def kernel(**inputs):
    out, _ = _run(inputs, trace=False)
    return out

